# revision 38
# baseline (speedup 1.0000x reference)
"""Decode-step multi-head attention with KV cache (DeepSpeed-inference style).

Full shapes (hardcoded per problem spec):
  query/key/value: [16, 1, 2048] f32
  key_cache/value_cache: [16, 16, 4096, 128] f32
  cache_len: scalar int (2048)
Output: [16, 1, 2048] f32

Strategy: data-parallel over batch across 8 NeuronCores (2 batches/core =
32 (batch, head) pairs per core). Per pair, the core streams the K and V
cache slices from HBM, computes scores with multiply+reduce on VectorE,
exp via ScalarE (fused row-sum for the softmax denominator), and runs the
PV reduction on TensorE with the probability column as the (tiny)
stationary weight and V f16 as the moving operand, so each pair's output
lands as a PSUM row [1, head_dim] at partition 0 and is normalized
in-stream into a flat row buffer ([1, npairs*hd]) emitted with one DMA.

DMA transport (the kernel is HBM/DMA-engine bound; K+V = 64MiB/core):
- K rides the sync (SP) HWDGE queue in f32; V rides the gpsimd SWDGE
  queue cast f32->f16 in flight. This HWDGE+SWDGE mix keeps each SDMA
  engine's packets at ~280ns (25+ GB/s read); all-HWDGE configs (one or
  two rings) cap every engine at ~21 GB/s.
- The SWDGE descriptor rings throttle SDMA engine 15 ~17% below its
  peers (ring AXI-port contention). Since partition->engine mapping is
  fixed (E15 serves partitions 92-95 and 124-127), the cache rows are
  split unevenly: those 8 partitions carry 13 row-chunks/pair, the other
  120 carry 16, balancing finish times. The leftover 24 rows/pair ride a
  batched "extras" sidecar: two setup DMAs + O(1) batched score/PV work
  + one small matmul per pair. Dead score slots are masked to -1e30
  before exp so they contribute nothing.
- The last two pairs' K tiles load early on the scalar (ACT) HWDGE ring,
  so after the final V packet only one PV matmul train + normalize + a
  16KB out DMA remain.
"""

import functools
import os
from contextlib import ExitStack

import numpy as np

import concourse.bacc as bacc
import concourse.bass as bass
import concourse.mybir as mybir
import concourse.tile as tile
from concourse import bass_utils

N_CORES = 8
P = 128  # partitions

# Partition groups: (base partition, n partitions, chunks per partition).
# SDMA engine 15 owns partitions 92-95 and 124-127; it runs ~17% slower
# when SWDGE is active, so its partitions get 13/16 of the rows.
GROUPS = [(0, 92, 16), (92, 4, 13), (96, 28, 16), (124, 4, 13)]
MAIN_ROWS = sum(np_ * cnt for _, np_, cnt in GROUPS)  # 2024
NCHT = max(cnt for _, _, cnt in GROUPS)               # 16 chunk slots

# test.py hooks: set TRACE=True before calling kernel() to collect a profile.
TRACE = False
TRACE_KWARGS = {}
LAST_RESULTS = None


def _build_program(bl: int, n_heads: int, max_seq: int, hd: int, cache_len: int):
    """Build + compile the per-core program. bl = local batch count."""
    npairs = bl * n_heads
    assert hd == P
    sm_scale = 1.0 / float(np.sqrt(hd))
    n_extra = cache_len - MAIN_ROWS  # rows handled by the batched sidecar
    assert 0 < n_extra <= P
    N_EARLY_K = 2 if npairs >= 8 else 0

    nc = bacc.Bacc("TRN2", target_bir_lowering=False, debug=False)
    f32 = mybir.dt.float32
    f16 = mybir.dt.float16

    kc = nc.dram_tensor("kc", [bl, n_heads, max_seq, hd], f32, kind="ExternalInput").ap()
    vc = nc.dram_tensor("vc", [bl, n_heads, max_seq, hd], f32, kind="ExternalInput").ap()
    q = nc.dram_tensor("q", [npairs, hd], f32, kind="ExternalInput").ap()
    kn = nc.dram_tensor("kn", [npairs, hd], f32, kind="ExternalInput").ap()
    vn = nc.dram_tensor("vn", [npairs, hd], f32, kind="ExternalInput").ap()
    ident = nc.dram_tensor("ident", [P, P], f32, kind="ExternalInput").ap()
    # 0 where a (partition, chunk) slot holds a real cache row, -1e30 on the
    # dead slots of the short partitions (engine ops can't address partition
    # ranges off 32-alignment, so masking rides a full-tile add)
    mask = nc.dram_tensor("mask", [P, NCHT], f32, kind="ExternalInput").ap()
    out = nc.dram_tensor("out", [npairs, hd], f32, kind="ExternalOutput").ap()

    with tile.TileContext(nc) as tc, ExitStack() as ctx:
        singles = ctx.enter_context(tc.tile_pool(name="singles", bufs=1))
        rows = ctx.enter_context(tc.tile_pool(name="rows", bufs=1))
        kbufs = int(os.environ.get("KBUFS", "7"))
        kpool = ctx.enter_context(tc.tile_pool(name="kpool", bufs=kbufs - 1))
        vpool = ctx.enter_context(tc.tile_pool(name="vpool", bufs=kbufs))
        early_k = ctx.enter_context(tc.tile_pool(name="early_k", bufs=1))
        k16pool = ctx.enter_context(tc.tile_pool(name="k16pool", bufs=2))
        ppool = ctx.enter_context(tc.tile_pool(name="ppool", bufs=2))
        stats = ctx.enter_context(tc.tile_pool(name="stats", bufs=4))
        psum_o = ctx.enter_context(tc.tile_pool(name="psum_o", bufs=3, space="PSUM"))
        psum_q = ctx.enter_context(tc.tile_pool(name="psum_q", bufs=2, space="PSUM"))
        psum_1 = ctx.enter_context(tc.tile_pool(name="psum_1", bufs=1, space="PSUM"))

        def emit_kv_dmas(p, src_tensor, tl, engine):
            b, h = divmod(p, n_heads)
            r0 = 0
            for p0, np_, cnt in GROUPS:
                src = src_tensor[b, h, r0 : r0 + np_ * cnt, :].rearrange(
                    "(p c) d -> p c d", c=cnt
                )
                engine.dma_start(out=tl[p0 : p0 + np_, 0:cnt, :], in_=src)
                r0 += np_ * cnt

        def emit_k(p, engine=None, pool=None):
            kt = (pool or kpool).tile(
                [P, NCHT, hd], f32, tag=f"kt{p}" if pool else "kt"
            )
            if pool is not None:
                nc.vector.memset(kt[64:P, 13:NCHT, :], 0.0)
            emit_kv_dmas(p, kc, kt, engine or nc.sync)
            return kt

        def emit_v(p):
            # SWDGE casts V f32->f16 in flight (free on the DMA read side;
            # f16 weights/moving keep the PE matmuls at full rate)
            vt = vpool.tile([P, NCHT, hd], f16, tag="vt")
            emit_kv_dmas(p, vc, vt, nc.gpsimd)
            return vt

        # Prime every stream-pool slot's dead region (short partitions,
        # chunks 13..16) with zeros ONCE: the DMAs never write there, stale
        # SBUF bits can be NaN, and NaN poisons both the masked exp (NaN +
        # -1e30) and the PE (0-weight x NaN = NaN). [64:128) covers both
        # short groups and keeps the 32-aligned partition base engines need.
        for _ in range(kbufs - 1):
            t = kpool.tile([P, NCHT, hd], f32, tag="kt")
            nc.vector.memset(t[64:P, 13:NCHT, :], 0.0)
        for _ in range(kbufs):
            t = vpool.tile([P, NCHT, hd], f16, tag="vt")
            nc.vector.memset(t[64:P, 13:NCHT, :], 0.0)

        # issue the first pairs' K loads before any setup traffic so the
        # sync ring's first instruction is a K DMA
        PRELOAD = min(3, npairs)
        kts = {p: emit_k(p) for p in range(PRELOAD)}
        vts = {p: emit_v(p) for p in range(PRELOAD)}

        ones_col = singles.tile([P, 1], f32)
        nc.vector.memset(ones_col, 1.0)

        # small setup loads lead the scalar (ACT) HWDGE ring
        def flat_row(t):
            return bass.AP(
                tensor=t.tensor, offset=t.offset, ap=[[0, 1], [1, npairs * hd]]
            )

        q_row = rows.tile([1, npairs * hd], f32, tag="row")
        nc.scalar.dma_start(out=q_row, in_=flat_row(q))
        vn_row = singles.tile([1, npairs * hd], f32)
        nc.scalar.dma_start(out=vn_row, in_=flat_row(vn))
        kn_all = singles.tile([npairs, hd], f32)
        nc.scalar.dma_start(out=kn_all, in_=kn)
        q_all = singles.tile([npairs, hd], f32)
        nc.scalar.dma_start(out=q_all, in_=q)
        ident_sb = singles.tile([P, P], f32)
        nc.scalar.dma_start(out=ident_sb, in_=ident)
        mask_sb = singles.tile([P, NCHT], f32)
        nc.scalar.dma_start(out=mask_sb, in_=mask)

        # extras sidecar loads: the 24 cache rows not covered by the main
        # tiles, for all pairs at once (scalar HWDGE ring, f32)
        ke_sb = singles.tile([npairs, n_extra, hd], f32)
        nc.scalar.dma_start(
            out=ke_sb,
            in_=kc[:, :, MAIN_ROWS:cache_len, :].rearrange("b h r d -> (b h) r d"),
        )
        ve_sb = singles.tile([n_extra, npairs, hd], f32)
        nc.scalar.dma_start(
            out=ve_sb,
            in_=vc[:, :, MAIN_ROWS:cache_len, :].rearrange("b h r d -> r (b h) d"),
        )

        # the last pairs' K tiles, pinned, near the head of the scalar ring
        for p in range(npairs - N_EARLY_K, npairs):
            kts[p] = emit_k(p, engine=nc.scalar, pool=early_k)

        # all queries broadcast to every partition, once, as a PE outer
        # product ones[1,128] x q_row[1,*]. f16 replicas feed the 16-bit
        # score path.
        ones_row = singles.tile([1, P], f32)
        nc.vector.memset(ones_row, 1.0)
        q_all_b = singles.tile([P, npairs, hd], f16)
        GPAIRS = 4  # pairs per chunk; 4*hd f32 = one 2KB PSUM bank
        for g in range(npairs // GPAIRS):
            qb_ps = psum_q.tile([P, GPAIRS, hd], f32, tag="qb")
            qb_2d = bass.AP(
                tensor=qb_ps.tensor,
                offset=qb_ps.offset,
                ap=[qb_ps.ap[0], [1, GPAIRS * hd]],
            )
            nc.tensor.matmul(
                qb_2d,
                lhsT=ones_row,
                rhs=q_row[0:1, g * GPAIRS * hd : (g + 1) * GPAIRS * hd],
                start=True,
                stop=True,
            )
            nc.scalar.copy(q_all_b[:, g * GPAIRS : (g + 1) * GPAIRS, :], qb_ps)

        # Softmax denominators, one column per pair (partition 0).
        lrow = psum_1.tile([1, npairs], f32, tag="l")
        # Normalized output rows, all on partition 0, emitted with one DMA
        # (reuses q_row's slot - setup reads are done before pair 0 ends).
        final_row = rows.tile([1, npairs * hd], f32, tag="row")

        def bcast(ap2d, nb):
            return bass.AP(
                tensor=ap2d.tensor,
                offset=ap2d.offset,
                ap=[ap2d.ap[0], [0, nb], ap2d.ap[1]],
            )

        # ---- new-token scores, batched over all pairs, ending in a
        # partition-0 row p_newT so each pair's PV group can start with a
        # 1x1-weight matmul (PE requires base partition 0/32/64) ----
        prod_new = singles.tile([npairs, hd], f32)
        nc.vector.tensor_mul(prod_new, kn_all, q_all)
        s_new = singles.tile([npairs, 1], f32)
        nc.vector.reduce_sum(s_new, prod_new, axis=mybir.AxisListType.X)
        s_newT_ps = psum_1.tile([1, npairs], f32, tag="snT")
        nc.tensor.matmul(
            s_newT_ps, lhsT=s_new, rhs=ident_sb[:npairs, :npairs], start=True, stop=True
        )
        p_newT = singles.tile([1, npairs], f32)
        nc.scalar.activation(
            out=p_newT,
            in_=s_newT_ps,
            func=mybir.ActivationFunctionType.Exp,
            scale=sm_scale,
        )

        # lrow starts as p_newT (one 1x1 matmul); the extras and each pair
        # then accumulate their denominators (start=False)
        nc.tensor.matmul(
            lrow, lhsT=ones_col[0:1, 0:1], rhs=p_newT, start=True, stop=True
        )

        # ---- extras sidecar: scores/exp for the 24 leftover rows of every
        # pair, batched. Runs while the first K tiles stream in. ----
        prod_e = singles.tile([npairs, n_extra, hd], f32)
        nc.vector.tensor_mul(prod_e, ke_sb, bcast(q_all, n_extra))
        s_e = singles.tile([npairs, n_extra], f32)
        nc.vector.reduce_sum(s_e, prod_e, axis=mybir.AxisListType.X)
        p_e = singles.tile([npairs, n_extra], f32)
        l_e = singles.tile([npairs, 1], f32)
        nc.scalar.activation(
            out=p_e,
            in_=s_e,
            func=mybir.ActivationFunctionType.Exp,
            scale=sm_scale,
            accum_out=l_e,
        )
        # extras denominators -> lrow (transpose-accumulate in one matmul)
        nc.tensor.matmul(
            lrow, lhsT=l_e, rhs=ident_sb[:npairs, :npairs], start=False, stop=True
        )
        # extras probabilities transposed to [n_extra, npairs] for the
        # per-pair PV matmuls
        pT_ps = psum_1.tile([n_extra, npairs], f32, tag="pT")
        nc.tensor.matmul(
            pT_ps, lhsT=p_e, rhs=ident_sb[:npairs, :npairs], start=True, stop=True
        )
        pT_sb = singles.tile([n_extra, npairs], f32)
        nc.scalar.copy(pT_sb, pT_ps)

        def emit_scores(p, kt):
            """Score block for pair p: f16 cast -> mul -> pairwise folds ->
            reduce -> mask dead slots -> exp (+denominator accum)."""
            kt16 = k16pool.tile([P, NCHT, hd], f16, tag="kt16")
            nc.scalar.copy(kt16, kt)
            prod = ppool.tile([P, NCHT, hd], f16, tag="prod")
            nc.vector.tensor_mul(prod, kt16, bcast(q_all_b[:, p, :], NCHT))
            fold1 = ppool.tile([P, NCHT, hd // 2], f16, tag="f1")
            nc.vector.tensor_add(fold1, prod[:, :, : hd // 2], prod[:, :, hd // 2 :])
            fold2 = ppool.tile([P, NCHT, hd // 4], f16, tag="f2")
            nc.vector.tensor_add(fold2, fold1[:, :, : hd // 4], fold1[:, :, hd // 4 :])
            s_tile = stats.tile([P, NCHT], f32, tag="s")
            nc.vector.reduce_sum(s_tile, fold2, axis=mybir.AxisListType.X)
            # mask the dead (short-partition) chunk slots so exp -> 0
            nc.vector.tensor_add(s_tile, s_tile, mask_sb)
            p_tile = stats.tile([P, NCHT], f16, tag="p")
            l_part = stats.tile([P, 1], f32, tag="l")
            nc.scalar.activation(
                out=p_tile,
                in_=s_tile,
                func=mybir.ActivationFunctionType.Exp,
                scale=sm_scale,
                accum_out=l_part,
            )
            return p_tile, l_part

        def emit_pair_compute(p, kt, vt):
            p_tile, l_part = emit_scores(p, kt)
            nc.tensor.matmul(
                lrow[0:1, p : p + 1], lhsT=ones_col, rhs=l_part, start=False, stop=True
            )
            acc_p = psum_o.tile([1, hd], f32, tag="acc")
            # new-token term starts the PV accumulation group
            nc.tensor.matmul(
                acc_p,
                lhsT=p_newT[0:1, p : p + 1],
                rhs=vn_row[0:1, p * hd : (p + 1) * hd],
                start=True,
                stop=False,
            )
            # extras term
            nc.tensor.matmul(
                acc_p,
                lhsT=pT_sb[:, p : p + 1],
                rhs=ve_sb[:, p, :],
                start=False,
                stop=False,
            )
            # main PV train: probability column stationary, V f16 moving
            for c in range(NCHT):
                nc.tensor.matmul(
                    acc_p,
                    lhsT=p_tile[:, c : c + 1],
                    rhs=vt[:, c, :],
                    start=False,
                    stop=(c == NCHT - 1),
                )
            # per-pair normalize straight out of PSUM into the output row
            # buffer (runs mid-stream for every pair but the last)
            recip_p = stats.tile([1, 1], f32, tag="r")
            nc.vector.reciprocal(recip_p, lrow[0:1, p : p + 1])
            nc.scalar.mul(final_row[0:1, p * hd : (p + 1) * hd], acc_p, mul=recip_p)

        for p in range(npairs):
            if p not in kts:
                kts[p] = emit_k(p)
            if p not in vts:
                vts[p] = emit_v(p)
            emit_pair_compute(p, kts[p], vts[p])

        # ---- emit: one 16KB DMA of all normalized rows ----
        out_flat = bass.AP(
            tensor=out.tensor, offset=out.offset, ap=[[0, 1], [1, npairs * hd]]
        )
        nc.scalar.dma_start(out=out_flat, in_=final_row)

    nc.compile()
    return nc


@functools.lru_cache(maxsize=4)
def _program(bl, n_heads, max_seq, hd, cache_len):
    return _build_program(bl, n_heads, max_seq, hd, cache_len)


def kernel(query, key, value, key_cache, value_cache, cache_len):
    global LAST_RESULTS
    query = np.asarray(query, dtype=np.float32)
    key = np.asarray(key, dtype=np.float32)
    value = np.asarray(value, dtype=np.float32)
    key_cache = np.asarray(key_cache, dtype=np.float32)
    value_cache = np.asarray(value_cache, dtype=np.float32)
    cache_len = int(cache_len)

    b_sz, q_len, d_model = query.shape
    _, n_heads, max_seq, hd = key_cache.shape
    assert q_len == 1 and d_model == n_heads * hd
    assert b_sz % N_CORES == 0
    bl = b_sz // N_CORES

    prog = _program(bl, n_heads, max_seq, hd, cache_len)

    ident = np.eye(P, dtype=np.float32)
    mask = np.zeros((P, NCHT), dtype=np.float32)
    for p0, np_, cnt in GROUPS:
        mask[p0 : p0 + np_, cnt:NCHT] = -1e30
    in_maps = []
    for i in range(N_CORES):
        sl = slice(i * bl, (i + 1) * bl)
        in_maps.append(
            {
                "kc": np.ascontiguousarray(key_cache[sl]),
                "vc": np.ascontiguousarray(value_cache[sl]),
                "q": np.ascontiguousarray(query[sl]).reshape(bl * n_heads, hd),
                "kn": np.ascontiguousarray(key[sl]).reshape(bl * n_heads, hd),
                "vn": np.ascontiguousarray(value[sl]).reshape(bl * n_heads, hd),
                "ident": ident,
                "mask": mask,
            }
        )

    try:
        res = bass_utils.run_bass_kernel_spmd(
            prog, in_maps, core_ids=list(range(N_CORES)), trace=TRACE, **TRACE_KWARGS
        )
    except Exception:
        # A previously crashed NeuronCore can leave the first execution
        # attempt failing with a transient runtime error; retry once.
        res = bass_utils.run_bass_kernel_spmd(
            prog, in_maps, core_ids=list(range(N_CORES)), trace=TRACE, **TRACE_KWARGS
        )
    LAST_RESULTS = res
    outs = [res.results[i]["out"].reshape(bl, q_len, d_model) for i in range(N_CORES)]
    return np.concatenate(outs, axis=0)


# revision 46
# speedup vs baseline: 1.9115x; 1.9115x over previous
"""Decode-step multi-head attention with KV cache (DeepSpeed-inference style).

Full shapes (hardcoded per problem spec):
  query/key/value: [16, 1, 2048] f32
  key_cache/value_cache: [16, 16, 4096, 128] f32
  cache_len: scalar int (2048)
Output: [16, 1, 2048] f32

Strategy: data-parallel over batch across 8 NeuronCores (2 batches/core =
32 (batch, head) pairs per core). Per pair, the core streams the K and V
cache slices from HBM, computes scores with multiply+reduce on VectorE,
exp via ScalarE (fused row-sum for the softmax denominator), and runs the
PV reduction on TensorE with the probability column as the (tiny)
stationary weight and V f16 as the moving operand, so each pair's output
lands as a PSUM row [1, head_dim] at partition 0 and is normalized
in-stream into a flat row buffer ([1, npairs*hd]) emitted with one DMA.

DMA transport (the kernel is HBM/DMA-engine bound; K+V = 64MiB/core):
- K rides the sync (SP) HWDGE queue in f32; V rides the gpsimd SWDGE
  queue cast f32->f16 in flight. This HWDGE+SWDGE mix keeps each SDMA
  engine's packets at ~280ns (25+ GB/s read); all-HWDGE configs (one or
  two rings) cap every engine at ~21 GB/s.
- The SWDGE descriptor rings throttle SDMA engine 15 ~17% below its
  peers (ring AXI-port contention). Since partition->engine mapping is
  fixed (E15 serves partitions 92-95 and 124-127), the cache rows are
  split unevenly: those 8 partitions carry 13 row-chunks/pair, the other
  120 carry 16, balancing finish times. The leftover 24 rows/pair ride a
  batched "extras" sidecar: two setup DMAs + O(1) batched score/PV work
  + one small matmul per pair. Dead score slots are masked to -1e30
  before exp so they contribute nothing.
- The last two pairs' K tiles load early on the scalar (ACT) HWDGE ring,
  so after the final V packet only one PV matmul train + normalize + a
  16KB out DMA remain.
"""

import functools
import os
from contextlib import ExitStack

import numpy as np

import concourse.bacc as bacc
import concourse.bass as bass
import concourse.mybir as mybir
import concourse.tile as tile
from concourse import bass_utils

N_CORES = 8
P = 128  # partitions

# A DMA's partitions map to SDMA engines RELATIVE to its base partition
# (engine = (p - base) // 8), so a full-128-partition DMA puts partitions
# 120-127 on engine 15 - which runs ~17% slower when SWDGE is active. Each
# tile therefore loads as two DMAs: A = all 128 partitions x chunks 0..13
# (engines 0-15), B = partitions 0..120 x chunks 13..16 (engines 0-14).
# E15 ends up with 13/16 of a normal share, matching its speed deficit.
NCHT = 16        # chunk slots per partition (tile row stripe = 16 rows)
NSHORT = 13      # chunks on the short partitions (120..127)
NFULL = 120      # partitions carrying all 16 chunks
MAIN_ROWS = 128 * NSHORT + NFULL * (NCHT - NSHORT)  # 2024

# test.py hooks: set TRACE=True before calling kernel() to collect a profile.
TRACE = False
TRACE_KWARGS = {}
LAST_RESULTS = None


def _build_program(bl: int, n_heads: int, max_seq: int, hd: int, cache_len: int):
    """Build + compile the per-core program. bl = local batch count."""
    npairs = bl * n_heads
    assert hd == P
    sm_scale = 1.0 / float(np.sqrt(hd))
    n_extra = cache_len - MAIN_ROWS  # rows handled by the batched sidecar
    assert 0 < n_extra <= P
    N_EARLY_K = 2 if npairs >= 8 else 0

    nc = bacc.Bacc("TRN2", target_bir_lowering=False, debug=False)
    f32 = mybir.dt.float32
    f16 = mybir.dt.float16

    kc = nc.dram_tensor("kc", [bl, n_heads, max_seq, hd], f32, kind="ExternalInput").ap()
    vc = nc.dram_tensor("vc", [bl, n_heads, max_seq, hd], f32, kind="ExternalInput").ap()
    q = nc.dram_tensor("q", [npairs, hd], f32, kind="ExternalInput").ap()
    kn = nc.dram_tensor("kn", [npairs, hd], f32, kind="ExternalInput").ap()
    vn = nc.dram_tensor("vn", [npairs, hd], f32, kind="ExternalInput").ap()
    ident = nc.dram_tensor("ident", [P, P], f32, kind="ExternalInput").ap()
    # 0 where a (partition, chunk) slot holds a real cache row, -1e30 on the
    # dead slots of the short partitions (engine ops can't address partition
    # ranges off 32-alignment, so masking rides a full-tile add)
    mask = nc.dram_tensor("mask", [P, NCHT], f32, kind="ExternalInput").ap()
    out = nc.dram_tensor("out", [npairs, hd], f32, kind="ExternalOutput").ap()

    with tile.TileContext(nc) as tc, ExitStack() as ctx:
        singles = ctx.enter_context(tc.tile_pool(name="singles", bufs=1))
        rows = ctx.enter_context(tc.tile_pool(name="rows", bufs=1))
        kbufs = int(os.environ.get("KBUFS", "7"))
        kpool = ctx.enter_context(tc.tile_pool(name="kpool", bufs=kbufs - 1))
        vpool = ctx.enter_context(tc.tile_pool(name="vpool", bufs=kbufs))
        early_k = ctx.enter_context(tc.tile_pool(name="early_k", bufs=1))
        k16pool = ctx.enter_context(tc.tile_pool(name="k16pool", bufs=2))
        ppool = ctx.enter_context(tc.tile_pool(name="ppool", bufs=2))
        stats = ctx.enter_context(tc.tile_pool(name="stats", bufs=4))
        psum_o = ctx.enter_context(tc.tile_pool(name="psum_o", bufs=3, space="PSUM"))
        psum_q = ctx.enter_context(tc.tile_pool(name="psum_q", bufs=2, space="PSUM"))
        psum_1 = ctx.enter_context(tc.tile_pool(name="psum_1", bufs=1, space="PSUM"))

        def emit_kv_dmas(p, src_tensor, tl, engine):
            # tile slot [p, c] <- cache row p*16 + c; two DMAs per tile
            b, h = divmod(p, n_heads)
            blk = src_tensor[b, h, 0:1, 0:1]  # AP anchored at the (b,h) block
            stripe = NCHT * hd  # 16 rows per partition
            a_src = bass.AP(
                tensor=blk.tensor,
                offset=blk.offset,
                ap=[[stripe, P], [hd, NSHORT], [1, hd]],
            )
            engine.dma_start(out=tl[:, 0:NSHORT, :], in_=a_src)
            b_src = bass.AP(
                tensor=blk.tensor,
                offset=blk.offset + NSHORT * hd,
                ap=[[stripe, NFULL], [hd, NCHT - NSHORT], [1, hd]],
            )
            engine.dma_start(out=tl[0:NFULL, NSHORT:NCHT, :], in_=b_src)

        def emit_k(p, engine=None, pool=None):
            kt = (pool or kpool).tile(
                [P, NCHT, hd], f32, tag=f"kt{p}" if pool else "kt"
            )
            if pool is not None:
                nc.vector.memset(kt[96:P, NSHORT:NCHT, :], 0.0)
            emit_kv_dmas(p, kc, kt, engine or nc.sync)
            return kt

        def emit_v(p):
            # SWDGE casts V f32->f16 in flight (free on the DMA read side;
            # f16 weights/moving keep the PE matmuls at full rate)
            vt = vpool.tile([P, NCHT, hd], f16, tag="vt")
            emit_kv_dmas(p, vc, vt, nc.gpsimd)
            return vt

        # Prime every stream-pool slot's dead region (short partitions,
        # chunks 13..16) with zeros ONCE: the DMAs never write there, stale
        # SBUF bits can be NaN, and NaN poisons both the masked exp (NaN +
        # -1e30) and the PE (0-weight x NaN = NaN). [64:128) covers both
        # short groups and keeps the 32-aligned partition base engines need.
        for _ in range(kbufs - 1):
            t = kpool.tile([P, NCHT, hd], f32, tag="kt")
            nc.vector.memset(t[96:P, NSHORT:NCHT, :], 0.0)
        for _ in range(kbufs):
            t = vpool.tile([P, NCHT, hd], f16, tag="vt")
            nc.vector.memset(t[96:P, NSHORT:NCHT, :], 0.0)

        # issue the first pairs' K loads before any setup traffic so the
        # sync ring's first instruction is a K DMA
        PRELOAD = min(3, npairs)
        kts = {p: emit_k(p) for p in range(PRELOAD)}
        vts = {p: emit_v(p) for p in range(PRELOAD)}

        ones_col = singles.tile([P, 1], f32)
        nc.vector.memset(ones_col, 1.0)

        # small setup loads lead the scalar (ACT) HWDGE ring
        def flat_row(t):
            return bass.AP(
                tensor=t.tensor, offset=t.offset, ap=[[0, 1], [1, npairs * hd]]
            )

        q_row = rows.tile([1, npairs * hd], f32, tag="row")
        nc.scalar.dma_start(out=q_row, in_=flat_row(q))
        vn_row = singles.tile([1, npairs * hd], f32)
        nc.scalar.dma_start(out=vn_row, in_=flat_row(vn))
        kn_all = singles.tile([npairs, hd], f32)
        nc.scalar.dma_start(out=kn_all, in_=kn)
        q_all = singles.tile([npairs, hd], f32)
        nc.scalar.dma_start(out=q_all, in_=q)
        ident_sb = singles.tile([P, P], f32)
        nc.scalar.dma_start(out=ident_sb, in_=ident)
        mask_sb = singles.tile([P, NCHT], f32)
        nc.scalar.dma_start(out=mask_sb, in_=mask)

        # extras sidecar loads: the 24 cache rows not covered by the main
        # tiles (rows p*16+c for p in 120..128, c in 13..16), for all pairs
        # at once (scalar HWDGE ring, f32). Extra row index r = p8*3 + c.
        n_p8 = P - NFULL
        n_c3 = NCHT - NSHORT
        stripe = NCHT * hd
        e_off = NFULL * stripe + NSHORT * hd
        bh_stride = max_seq * hd
        # extra row index r = c3*8 + p8 (one 3-dim DMA per c3 value)
        ke_sb = singles.tile([npairs, n_extra, hd], f32)
        ve_sb = singles.tile([n_extra, npairs, hd], f32)
        for c in range(n_c3):
            ke_src = bass.AP(
                tensor=kc.tensor,
                offset=kc.offset + e_off + c * hd,
                ap=[[bh_stride, npairs], [stripe, n_p8], [1, hd]],
            )
            nc.scalar.dma_start(
                out=ke_sb[:, c * n_p8 : (c + 1) * n_p8, :], in_=ke_src
            )
            ve_src = bass.AP(
                tensor=vc.tensor,
                offset=vc.offset + e_off + c * hd,
                ap=[[stripe, n_p8], [bh_stride, npairs], [1, hd]],
            )
            nc.scalar.dma_start(
                out=ve_sb[c * n_p8 : (c + 1) * n_p8, :, :], in_=ve_src
            )

        # the last pairs' K tiles, pinned, near the head of the scalar ring
        for p in range(npairs - N_EARLY_K, npairs):
            kts[p] = emit_k(p, engine=nc.scalar, pool=early_k)

        # all queries broadcast to every partition, once, as a PE outer
        # product ones[1,128] x q_row[1,*]. f16 replicas feed the 16-bit
        # score path.
        ones_row = singles.tile([1, P], f32)
        nc.vector.memset(ones_row, 1.0)
        q_all_b = singles.tile([P, npairs, hd], f16)
        GPAIRS = 4  # pairs per chunk; 4*hd f32 = one 2KB PSUM bank
        for g in range(npairs // GPAIRS):
            qb_ps = psum_q.tile([P, GPAIRS, hd], f32, tag="qb")
            qb_2d = bass.AP(
                tensor=qb_ps.tensor,
                offset=qb_ps.offset,
                ap=[qb_ps.ap[0], [1, GPAIRS * hd]],
            )
            nc.tensor.matmul(
                qb_2d,
                lhsT=ones_row,
                rhs=q_row[0:1, g * GPAIRS * hd : (g + 1) * GPAIRS * hd],
                start=True,
                stop=True,
            )
            nc.scalar.copy(q_all_b[:, g * GPAIRS : (g + 1) * GPAIRS, :], qb_ps)

        # Softmax denominators, one column per pair (partition 0).
        lrow = psum_1.tile([1, npairs], f32, tag="l")
        # Normalized output rows, all on partition 0, emitted with one DMA
        # (reuses q_row's slot - setup reads are done before pair 0 ends).
        final_row = rows.tile([1, npairs * hd], f32, tag="row")

        def bcast(ap2d, nb):
            return bass.AP(
                tensor=ap2d.tensor,
                offset=ap2d.offset,
                ap=[ap2d.ap[0], [0, nb], ap2d.ap[1]],
            )

        # ---- new-token scores, batched over all pairs, ending in a
        # partition-0 row p_newT so each pair's PV group can start with a
        # 1x1-weight matmul (PE requires base partition 0/32/64) ----
        prod_new = singles.tile([npairs, hd], f32)
        nc.vector.tensor_mul(prod_new, kn_all, q_all)
        s_new = singles.tile([npairs, 1], f32)
        nc.vector.reduce_sum(s_new, prod_new, axis=mybir.AxisListType.X)
        s_newT_ps = psum_1.tile([1, npairs], f32, tag="snT")
        nc.tensor.matmul(
            s_newT_ps, lhsT=s_new, rhs=ident_sb[:npairs, :npairs], start=True, stop=True
        )
        p_newT = singles.tile([1, npairs], f32)
        nc.scalar.activation(
            out=p_newT,
            in_=s_newT_ps,
            func=mybir.ActivationFunctionType.Exp,
            scale=sm_scale,
        )

        # lrow starts as p_newT (one 1x1 matmul); the extras and each pair
        # then accumulate their denominators (start=False)
        nc.tensor.matmul(
            lrow, lhsT=ones_col[0:1, 0:1], rhs=p_newT, start=True, stop=True
        )

        # ---- extras sidecar: scores/exp for the 24 leftover rows of every
        # pair, batched. Runs while the first K tiles stream in. ----
        prod_e = singles.tile([npairs, n_extra, hd], f32)
        nc.vector.tensor_mul(prod_e, ke_sb, bcast(q_all, n_extra))
        s_e = singles.tile([npairs, n_extra], f32)
        nc.vector.reduce_sum(s_e, prod_e, axis=mybir.AxisListType.X)
        p_e = singles.tile([npairs, n_extra], f32)
        l_e = singles.tile([npairs, 1], f32)
        nc.scalar.activation(
            out=p_e,
            in_=s_e,
            func=mybir.ActivationFunctionType.Exp,
            scale=sm_scale,
            accum_out=l_e,
        )
        # extras denominators -> lrow (transpose-accumulate in one matmul)
        nc.tensor.matmul(
            lrow, lhsT=l_e, rhs=ident_sb[:npairs, :npairs], start=False, stop=True
        )
        # extras probabilities transposed to [n_extra, npairs] for the
        # per-pair PV matmuls
        pT_ps = psum_1.tile([n_extra, npairs], f32, tag="pT")
        nc.tensor.matmul(
            pT_ps, lhsT=p_e, rhs=ident_sb[:npairs, :npairs], start=True, stop=True
        )
        pT_sb = singles.tile([n_extra, npairs], f32)
        nc.scalar.copy(pT_sb, pT_ps)

        def emit_scores(p, kt):
            """Score block for pair p: f16 cast -> mul -> pairwise folds ->
            reduce -> mask dead slots -> exp (+denominator accum)."""
            kt16 = k16pool.tile([P, NCHT, hd], f16, tag="kt16")
            nc.scalar.copy(kt16, kt)
            prod = ppool.tile([P, NCHT, hd], f16, tag="prod")
            nc.vector.tensor_mul(prod, kt16, bcast(q_all_b[:, p, :], NCHT))
            fold1 = ppool.tile([P, NCHT, hd // 2], f16, tag="f1")
            nc.vector.tensor_add(fold1, prod[:, :, : hd // 2], prod[:, :, hd // 2 :])
            fold2 = ppool.tile([P, NCHT, hd // 4], f16, tag="f2")
            nc.vector.tensor_add(fold2, fold1[:, :, : hd // 4], fold1[:, :, hd // 4 :])
            s_tile = stats.tile([P, NCHT], f32, tag="s")
            nc.vector.reduce_sum(s_tile, fold2, axis=mybir.AxisListType.X)
            # mask the dead (short-partition) chunk slots so exp -> 0
            nc.vector.tensor_add(s_tile, s_tile, mask_sb)
            p_tile = stats.tile([P, NCHT], f16, tag="p")
            l_part = stats.tile([P, 1], f32, tag="l")
            nc.scalar.activation(
                out=p_tile,
                in_=s_tile,
                func=mybir.ActivationFunctionType.Exp,
                scale=sm_scale,
                accum_out=l_part,
            )
            return p_tile, l_part

        def emit_pair_compute(p, kt, vt):
            p_tile, l_part = emit_scores(p, kt)
            nc.tensor.matmul(
                lrow[0:1, p : p + 1], lhsT=ones_col, rhs=l_part, start=False, stop=True
            )
            acc_p = psum_o.tile([1, hd], f32, tag="acc")
            # new-token term starts the PV accumulation group
            nc.tensor.matmul(
                acc_p,
                lhsT=p_newT[0:1, p : p + 1],
                rhs=vn_row[0:1, p * hd : (p + 1) * hd],
                start=True,
                stop=False,
            )
            # extras term
            nc.tensor.matmul(
                acc_p,
                lhsT=pT_sb[:, p : p + 1],
                rhs=ve_sb[:, p, :],
                start=False,
                stop=False,
            )
            # main PV train: probability column stationary, V f16 moving
            for c in range(NCHT):
                nc.tensor.matmul(
                    acc_p,
                    lhsT=p_tile[:, c : c + 1],
                    rhs=vt[:, c, :],
                    start=False,
                    stop=(c == NCHT - 1),
                )
            # per-pair normalize straight out of PSUM into the output row
            # buffer (runs mid-stream for every pair but the last)
            recip_p = stats.tile([1, 1], f32, tag="r")
            nc.vector.reciprocal(recip_p, lrow[0:1, p : p + 1])
            nc.scalar.mul(final_row[0:1, p * hd : (p + 1) * hd], acc_p, mul=recip_p)

        for p in range(npairs):
            if p not in kts:
                kts[p] = emit_k(p)
            if p not in vts:
                vts[p] = emit_v(p)
            emit_pair_compute(p, kts[p], vts[p])

        # ---- emit: one 16KB DMA of all normalized rows ----
        out_flat = bass.AP(
            tensor=out.tensor, offset=out.offset, ap=[[0, 1], [1, npairs * hd]]
        )
        nc.scalar.dma_start(out=out_flat, in_=final_row)

    nc.compile()
    return nc


@functools.lru_cache(maxsize=4)
def _program(bl, n_heads, max_seq, hd, cache_len):
    return _build_program(bl, n_heads, max_seq, hd, cache_len)


def kernel(query, key, value, key_cache, value_cache, cache_len):
    global LAST_RESULTS
    query = np.asarray(query, dtype=np.float32)
    key = np.asarray(key, dtype=np.float32)
    value = np.asarray(value, dtype=np.float32)
    key_cache = np.asarray(key_cache, dtype=np.float32)
    value_cache = np.asarray(value_cache, dtype=np.float32)
    cache_len = int(cache_len)

    b_sz, q_len, d_model = query.shape
    _, n_heads, max_seq, hd = key_cache.shape
    assert q_len == 1 and d_model == n_heads * hd
    assert b_sz % N_CORES == 0
    bl = b_sz // N_CORES

    prog = _program(bl, n_heads, max_seq, hd, cache_len)

    ident = np.eye(P, dtype=np.float32)
    mask = np.zeros((P, NCHT), dtype=np.float32)
    mask[NFULL:P, NSHORT:NCHT] = -1e30
    in_maps = []
    for i in range(N_CORES):
        sl = slice(i * bl, (i + 1) * bl)
        in_maps.append(
            {
                "kc": np.ascontiguousarray(key_cache[sl]),
                "vc": np.ascontiguousarray(value_cache[sl]),
                "q": np.ascontiguousarray(query[sl]).reshape(bl * n_heads, hd),
                "kn": np.ascontiguousarray(key[sl]).reshape(bl * n_heads, hd),
                "vn": np.ascontiguousarray(value[sl]).reshape(bl * n_heads, hd),
                "ident": ident,
                "mask": mask,
            }
        )

    try:
        res = bass_utils.run_bass_kernel_spmd(
            prog, in_maps, core_ids=list(range(N_CORES)), trace=TRACE, **TRACE_KWARGS
        )
    except Exception:
        # A previously crashed NeuronCore can leave the first execution
        # attempt failing with a transient runtime error; retry once.
        res = bass_utils.run_bass_kernel_spmd(
            prog, in_maps, core_ids=list(range(N_CORES)), trace=TRACE, **TRACE_KWARGS
        )
    LAST_RESULTS = res
    outs = [res.results[i]["out"].reshape(bl, q_len, d_model) for i in range(N_CORES)]
    return np.concatenate(outs, axis=0)


# revision 47
# speedup vs baseline: 2.0379x; 1.0661x over previous
"""Decode-step multi-head attention with KV cache (DeepSpeed-inference style).

Full shapes (hardcoded per problem spec):
  query/key/value: [16, 1, 2048] f32
  key_cache/value_cache: [16, 16, 4096, 128] f32
  cache_len: scalar int (2048)
Output: [16, 1, 2048] f32

Strategy: data-parallel over batch across 8 NeuronCores (2 batches/core =
32 (batch, head) pairs per core). Per pair, the core streams the K and V
cache slices from HBM, computes scores with multiply+reduce on VectorE,
exp via ScalarE (fused row-sum for the softmax denominator), and runs the
PV reduction on TensorE with the probability column as the (tiny)
stationary weight and V f16 as the moving operand, so each pair's output
lands as a PSUM row [1, head_dim] at partition 0 and is normalized
in-stream into a flat row buffer ([1, npairs*hd]) emitted with one DMA.

DMA transport (the kernel is DMA-engine bound; K+V = 64MiB/core):
- K rides the sync (SP) HWDGE queue in f32; V rides the gpsimd SWDGE
  queue cast f32->f16 in flight. This HWDGE+SWDGE mix keeps each SDMA
  engine's packets at ~280ns (~25 GB/s read); all-HWDGE configs cap
  every engine at ~21 GB/s regardless of ring count.
- The SWDGE descriptor rings throttle SDMA engine 15 ~17% below its
  peers. A DMA's partitions map to engines relative to its base
  (engine = partition//8 for full-width DMAs), so E15 serves partitions
  120-127. To offload it WITHOUT shrinking DMA lines (packet efficiency
  dies below ~8KB per partition line), 3 of every 16 pairs load as
  [120 partitions x 17 chunks] tiles (engines 0-14 only, 8.7KB lines);
  the rest are standard [128 x 16]. E15 ends up with 13/16 of a normal
  share, matching its speed deficit, and every DMA keeps full-size
  lines. The reduced pairs' 8 leftover cache rows ride a tiny batched
  sidecar (setup DMAs + one 8-deep matmul per reduced pair).
- The last pair's K tile loads early on the scalar (ACT) HWDGE ring, so
  after the final V packet only one PV matmul train + normalize + a
  16KB out DMA remain.
"""

import functools
import os
from contextlib import ExitStack

import numpy as np

import concourse.bacc as bacc
import concourse.bass as bass
import concourse.mybir as mybir
import concourse.tile as tile
from concourse import bass_utils

N_CORES = 8
P = 128   # partitions
NCH_S = 16   # chunks/partition, standard pairs (rows = p*16 + c)
NP_R = 120   # partitions used by reduced pairs
NCH_R = 17   # chunks/partition, reduced pairs (rows = p*17 + c)
# pairs whose tiles skip engine 15 (3 of every 16)
RED_MOD = (5, 10, 15)

# test.py hooks: set TRACE=True before calling kernel() to collect a profile.
TRACE = False
TRACE_KWARGS = {}
LAST_RESULTS = None


def _reduced(npairs):
    return [p for p in range(npairs) if p % 16 in RED_MOD]


def _build_program(bl: int, n_heads: int, max_seq: int, hd: int, cache_len: int):
    """Build + compile the per-core program. bl = local batch count."""
    npairs = bl * n_heads
    assert hd == P
    assert cache_len == P * NCH_S == NP_R * NCH_R + 8
    sm_scale = 1.0 / float(np.sqrt(hd))
    reduced = _reduced(npairs)
    n_red = len(reduced)
    n_extra = cache_len - NP_R * NCH_R  # leftover rows per reduced pair (8)

    nc = bacc.Bacc("TRN2", target_bir_lowering=False, debug=False)
    f32 = mybir.dt.float32
    f16 = mybir.dt.float16

    kc = nc.dram_tensor("kc", [bl, n_heads, max_seq, hd], f32, kind="ExternalInput").ap()
    vc = nc.dram_tensor("vc", [bl, n_heads, max_seq, hd], f32, kind="ExternalInput").ap()
    q = nc.dram_tensor("q", [npairs, hd], f32, kind="ExternalInput").ap()
    kn = nc.dram_tensor("kn", [npairs, hd], f32, kind="ExternalInput").ap()
    vn = nc.dram_tensor("vn", [npairs, hd], f32, kind="ExternalInput").ap()
    ident = nc.dram_tensor("ident", [P, P], f32, kind="ExternalInput").ap()
    # sel[i, j] = 1 where j == reduced[i]: scatters the sidecar's
    # denominators into lrow columns with one matmul
    sel = nc.dram_tensor("sel", [n_red, npairs], f32, kind="ExternalInput").ap()
    out = nc.dram_tensor("out", [npairs, hd], f32, kind="ExternalOutput").ap()

    bh_stride = max_seq * hd

    with tile.TileContext(nc) as tc, ExitStack() as ctx:
        singles = ctx.enter_context(tc.tile_pool(name="singles", bufs=1))
        rows = ctx.enter_context(tc.tile_pool(name="rows", bufs=1))
        kpool = ctx.enter_context(tc.tile_pool(name="kpool", bufs=5))
        vpool = ctx.enter_context(tc.tile_pool(name="vpool", bufs=6))
        rkpool = ctx.enter_context(tc.tile_pool(name="rkpool", bufs=2))
        rvpool = ctx.enter_context(tc.tile_pool(name="rvpool", bufs=2))
        early_k = ctx.enter_context(tc.tile_pool(name="early_k", bufs=1))
        k16pool = ctx.enter_context(tc.tile_pool(name="k16pool", bufs=2))
        ppool = ctx.enter_context(tc.tile_pool(name="ppool", bufs=2))
        rppool = ctx.enter_context(tc.tile_pool(name="rppool", bufs=1))
        stats = ctx.enter_context(tc.tile_pool(name="stats", bufs=4))
        psum_o = ctx.enter_context(tc.tile_pool(name="psum_o", bufs=3, space="PSUM"))
        psum_q = ctx.enter_context(tc.tile_pool(name="psum_q", bufs=2, space="PSUM"))
        psum_1 = ctx.enter_context(tc.tile_pool(name="psum_1", bufs=1, space="PSUM"))

        def shape_of(p):
            return (NP_R, NCH_R) if p % 16 in RED_MOD else (P, NCH_S)

        def emit_k(p, engine=None, pool=None):
            np_, nch = shape_of(p)
            b, h = divmod(p, n_heads)
            kt = (pool or (rkpool if nch == NCH_R else kpool)).tile(
                [np_, nch, hd], f32, tag=f"kt{p}" if pool else "kt"
            )
            src = kc[b, h, 0 : np_ * nch, :].rearrange("(p c) d -> p c d", c=nch)
            (engine or nc.sync).dma_start(out=kt, in_=src)
            return kt

        def emit_v(p):
            # SWDGE casts V f32->f16 in flight (free on the DMA read side;
            # f16 weights/moving keep the PE matmuls at full rate)
            np_, nch = shape_of(p)
            b, h = divmod(p, n_heads)
            vt = (rvpool if nch == NCH_R else vpool).tile(
                [np_, nch, hd], f16, tag="vt"
            )
            src = vc[b, h, 0 : np_ * nch, :].rearrange("(p c) d -> p c d", c=nch)
            nc.gpsimd.dma_start(out=vt, in_=src)
            return vt

        # issue the first pairs' K loads before any setup traffic so the
        # sync ring's first instruction is a K DMA
        PRELOAD = min(3, npairs)
        kts = {p: emit_k(p) for p in range(PRELOAD)}
        vts = {p: emit_v(p) for p in range(PRELOAD)}

        ones_col = singles.tile([P, 1], f32)
        nc.vector.memset(ones_col, 1.0)

        # small setup loads lead the scalar (ACT) HWDGE ring
        def flat_row(t):
            return bass.AP(
                tensor=t.tensor, offset=t.offset, ap=[[0, 1], [1, npairs * hd]]
            )

        q_row = rows.tile([1, npairs * hd], f32, tag="row")
        nc.scalar.dma_start(out=q_row, in_=flat_row(q))
        vn_row = singles.tile([1, npairs * hd], f32)
        nc.scalar.dma_start(out=vn_row, in_=flat_row(vn))
        kn_all = singles.tile([npairs, hd], f32)
        nc.scalar.dma_start(out=kn_all, in_=kn)
        q_all = singles.tile([npairs, hd], f32)
        nc.scalar.dma_start(out=q_all, in_=q)
        ident_sb = singles.tile([P, P], f32)
        nc.scalar.dma_start(out=ident_sb, in_=ident)
        sel_sb = singles.tile([n_red, npairs], f32)
        nc.scalar.dma_start(out=sel_sb, in_=sel)

        # sidecar loads: the reduced pairs' leftover rows (2040..2047) and
        # their q rows, batched per block-of-16-pairs (uniform stride)
        e_row0 = NP_R * NCH_R
        ke2 = singles.tile([n_red, n_extra, hd], f32)
        ve2 = singles.tile([n_extra, n_red, hd], f32)
        q2 = singles.tile([n_red, hd], f32)
        nblk = npairs // 16
        per_blk = len(RED_MOD)
        for blk in range(nblk):
            i0 = blk * per_blk
            pair0 = blk * 16 + RED_MOD[0]
            pstride = RED_MOD[1] - RED_MOD[0]
            base = pair0 * bh_stride + e_row0 * hd
            nc.scalar.dma_start(
                out=ke2[i0 : i0 + per_blk, :, :],
                in_=bass.AP(
                    tensor=kc.tensor,
                    offset=kc.offset + base,
                    ap=[[pstride * bh_stride, per_blk], [hd, n_extra], [1, hd]],
                ),
            )
            nc.scalar.dma_start(
                out=ve2[:, i0 : i0 + per_blk, :],
                in_=bass.AP(
                    tensor=vc.tensor,
                    offset=vc.offset + base,
                    ap=[[hd, n_extra], [pstride * bh_stride, per_blk], [1, hd]],
                ),
            )
            nc.scalar.dma_start(
                out=q2[i0 : i0 + per_blk, :],
                in_=bass.AP(
                    tensor=q.tensor,
                    offset=q.offset + pair0 * hd,
                    ap=[[pstride * hd, per_blk], [1, hd]],
                ),
            )

        # the last pair's K tile, pinned, near the head of the scalar ring
        N_EARLY_K = 1 if npairs >= 8 else 0
        for p in range(npairs - N_EARLY_K, npairs):
            kts[p] = emit_k(p, engine=nc.scalar, pool=early_k)

        # all queries broadcast to every partition, once, as a PE outer
        # product ones[1,128] x q_row[1,*]. f16 replicas feed the 16-bit
        # score path.
        ones_row = singles.tile([1, P], f32)
        nc.vector.memset(ones_row, 1.0)
        q_all_b = singles.tile([P, npairs, hd], f16)
        GPAIRS = 4  # pairs per chunk; 4*hd f32 = one 2KB PSUM bank
        for g in range(npairs // GPAIRS):
            qb_ps = psum_q.tile([P, GPAIRS, hd], f32, tag="qb")
            qb_2d = bass.AP(
                tensor=qb_ps.tensor,
                offset=qb_ps.offset,
                ap=[qb_ps.ap[0], [1, GPAIRS * hd]],
            )
            nc.tensor.matmul(
                qb_2d,
                lhsT=ones_row,
                rhs=q_row[0:1, g * GPAIRS * hd : (g + 1) * GPAIRS * hd],
                start=True,
                stop=True,
            )
            nc.scalar.copy(q_all_b[:, g * GPAIRS : (g + 1) * GPAIRS, :], qb_ps)

        # Softmax denominators, one column per pair (partition 0).
        lrow = psum_1.tile([1, npairs], f32, tag="l")
        # Normalized output rows, all on partition 0, emitted with one DMA
        # (reuses q_row's slot - setup reads are done before pair 0 ends).
        final_row = rows.tile([1, npairs * hd], f32, tag="row")

        def bcast(ap2d, nb):
            return bass.AP(
                tensor=ap2d.tensor,
                offset=ap2d.offset,
                ap=[ap2d.ap[0], [0, nb], ap2d.ap[1]],
            )

        # ---- new-token scores, batched over all pairs, ending in a
        # partition-0 row p_newT so each pair's PV group can start with a
        # 1x1-weight matmul (PE requires base partition 0/32/64) ----
        prod_new = singles.tile([npairs, hd], f32)
        nc.vector.tensor_mul(prod_new, kn_all, q_all)
        s_new = singles.tile([npairs, 1], f32)
        nc.vector.reduce_sum(s_new, prod_new, axis=mybir.AxisListType.X)
        s_newT_ps = psum_1.tile([1, npairs], f32, tag="snT")
        nc.tensor.matmul(
            s_newT_ps, lhsT=s_new, rhs=ident_sb[:npairs, :npairs], start=True, stop=True
        )
        p_newT = singles.tile([1, npairs], f32)
        nc.scalar.activation(
            out=p_newT,
            in_=s_newT_ps,
            func=mybir.ActivationFunctionType.Exp,
            scale=sm_scale,
        )

        # lrow starts as p_newT (one 1x1 matmul); the sidecar and each pair
        # then accumulate their denominators (start=False)
        nc.tensor.matmul(
            lrow, lhsT=ones_col[0:1, 0:1], rhs=p_newT, start=True, stop=True
        )

        # ---- sidecar: scores/exp for the reduced pairs' leftover rows,
        # batched. Runs while the first K tiles stream in. ----
        prod_e = singles.tile([n_red, n_extra, hd], f32)
        nc.vector.tensor_mul(prod_e, ke2, bcast(q2, n_extra))
        s_e = singles.tile([n_red, n_extra], f32)
        nc.vector.reduce_sum(s_e, prod_e, axis=mybir.AxisListType.X)
        p_e = singles.tile([n_red, n_extra], f32)
        l_e = singles.tile([n_red, 1], f32)
        nc.scalar.activation(
            out=p_e,
            in_=s_e,
            func=mybir.ActivationFunctionType.Exp,
            scale=sm_scale,
            accum_out=l_e,
        )
        # sidecar denominators -> lrow columns (scatter-accumulate matmul)
        nc.tensor.matmul(lrow, lhsT=l_e, rhs=sel_sb, start=False, stop=True)
        # sidecar probabilities transposed to [n_extra, n_red] for the
        # per-pair PV matmuls
        pT_ps = psum_1.tile([n_extra, n_red], f32, tag="pT")
        nc.tensor.matmul(
            pT_ps, lhsT=p_e, rhs=ident_sb[:n_red, :n_red], start=True, stop=True
        )
        pT_sb = singles.tile([n_extra, n_red], f32)
        nc.scalar.copy(pT_sb, pT_ps)

        def emit_pair_compute(p, kt, vt):
            np_, nch = shape_of(p)
            red = nch == NCH_R
            sfx = "r" if red else ""
            pp = rppool if red else ppool
            # scores: f16 cast -> mul -> pairwise folds -> reduce -> exp
            kt16 = k16pool.tile([np_, nch, hd], f16, tag="kt16" + sfx)
            nc.scalar.copy(kt16, kt)
            prod = pp.tile([np_, nch, hd], f16, tag="prod" + sfx)
            nc.vector.tensor_mul(prod, kt16, bcast(q_all_b[:np_, p, :], nch))
            fold1 = pp.tile([np_, nch, hd // 2], f16, tag="f1" + sfx)
            nc.vector.tensor_add(fold1, prod[:, :, : hd // 2], prod[:, :, hd // 2 :])
            fold2 = pp.tile([np_, nch, hd // 4], f16, tag="f2" + sfx)
            nc.vector.tensor_add(fold2, fold1[:, :, : hd // 4], fold1[:, :, hd // 4 :])
            s_tile = stats.tile([np_, nch], f32, tag="s" + sfx)
            nc.vector.reduce_sum(s_tile, fold2, axis=mybir.AxisListType.X)
            p_tile = stats.tile([np_, nch], f16, tag="p" + sfx)
            l_part = stats.tile([np_, 1], f32, tag="l" + sfx)
            nc.scalar.activation(
                out=p_tile,
                in_=s_tile,
                func=mybir.ActivationFunctionType.Exp,
                scale=sm_scale,
                accum_out=l_part,
            )
            nc.tensor.matmul(
                lrow[0:1, p : p + 1],
                lhsT=ones_col[:np_, :],
                rhs=l_part,
                start=False,
                stop=True,
            )
            acc_p = psum_o.tile([1, hd], f32, tag="acc")
            # new-token term starts the PV accumulation group
            nc.tensor.matmul(
                acc_p,
                lhsT=p_newT[0:1, p : p + 1],
                rhs=vn_row[0:1, p * hd : (p + 1) * hd],
                start=True,
                stop=False,
            )
            if red:
                i = reduced.index(p)
                nc.tensor.matmul(
                    acc_p,
                    lhsT=pT_sb[:, i : i + 1],
                    rhs=ve2[:, i, :],
                    start=False,
                    stop=False,
                )
            # main PV train: probability column stationary, V f16 moving
            for c in range(nch):
                nc.tensor.matmul(
                    acc_p,
                    lhsT=p_tile[:, c : c + 1],
                    rhs=vt[:, c, :],
                    start=False,
                    stop=(c == nch - 1),
                )
            # per-pair normalize straight out of PSUM into the output row
            # buffer (runs mid-stream for every pair but the last)
            recip_p = stats.tile([1, 1], f32, tag="r")
            nc.vector.reciprocal(recip_p, lrow[0:1, p : p + 1])
            nc.scalar.mul(final_row[0:1, p * hd : (p + 1) * hd], acc_p, mul=recip_p)

        for p in range(npairs):
            if p not in kts:
                kts[p] = emit_k(p)
            if p not in vts:
                vts[p] = emit_v(p)
            emit_pair_compute(p, kts[p], vts[p])

        # ---- emit: one 16KB DMA of all normalized rows ----
        out_flat = bass.AP(
            tensor=out.tensor, offset=out.offset, ap=[[0, 1], [1, npairs * hd]]
        )
        nc.scalar.dma_start(out=out_flat, in_=final_row)

    nc.compile()
    return nc


@functools.lru_cache(maxsize=4)
def _program(bl, n_heads, max_seq, hd, cache_len):
    return _build_program(bl, n_heads, max_seq, hd, cache_len)


def kernel(query, key, value, key_cache, value_cache, cache_len):
    global LAST_RESULTS
    query = np.asarray(query, dtype=np.float32)
    key = np.asarray(key, dtype=np.float32)
    value = np.asarray(value, dtype=np.float32)
    key_cache = np.asarray(key_cache, dtype=np.float32)
    value_cache = np.asarray(value_cache, dtype=np.float32)
    cache_len = int(cache_len)

    b_sz, q_len, d_model = query.shape
    _, n_heads, max_seq, hd = key_cache.shape
    assert q_len == 1 and d_model == n_heads * hd
    assert b_sz % N_CORES == 0
    bl = b_sz // N_CORES

    prog = _program(bl, n_heads, max_seq, hd, cache_len)

    npairs = bl * n_heads
    reduced = _reduced(npairs)
    ident = np.eye(P, dtype=np.float32)
    sel = np.zeros((len(reduced), npairs), dtype=np.float32)
    for i, p in enumerate(reduced):
        sel[i, p] = 1.0
    in_maps = []
    for i in range(N_CORES):
        sl = slice(i * bl, (i + 1) * bl)
        in_maps.append(
            {
                "kc": np.ascontiguousarray(key_cache[sl]),
                "vc": np.ascontiguousarray(value_cache[sl]),
                "q": np.ascontiguousarray(query[sl]).reshape(bl * n_heads, hd),
                "kn": np.ascontiguousarray(key[sl]).reshape(bl * n_heads, hd),
                "vn": np.ascontiguousarray(value[sl]).reshape(bl * n_heads, hd),
                "ident": ident,
                "sel": sel,
            }
        )

    try:
        res = bass_utils.run_bass_kernel_spmd(
            prog, in_maps, core_ids=list(range(N_CORES)), trace=TRACE, **TRACE_KWARGS
        )
    except Exception:
        # A previously crashed NeuronCore can leave the first execution
        # attempt failing with a transient runtime error; retry once.
        res = bass_utils.run_bass_kernel_spmd(
            prog, in_maps, core_ids=list(range(N_CORES)), trace=TRACE, **TRACE_KWARGS
        )
    LAST_RESULTS = res
    outs = [res.results[i]["out"].reshape(bl, q_len, d_model) for i in range(N_CORES)]
    return np.concatenate(outs, axis=0)


# revision 49
# speedup vs baseline: 2.0413x; 1.0017x over previous
"""Decode-step multi-head attention with KV cache (DeepSpeed-inference style).

Full shapes (hardcoded per problem spec):
  query/key/value: [16, 1, 2048] f32
  key_cache/value_cache: [16, 16, 4096, 128] f32
  cache_len: scalar int (2048)
Output: [16, 1, 2048] f32

Strategy: data-parallel over batch across 8 NeuronCores (2 batches/core =
32 (batch, head) pairs per core). Per pair, the core streams the K and V
cache slices from HBM, computes scores with multiply+reduce on VectorE,
exp via ScalarE (fused row-sum for the softmax denominator), and runs the
PV reduction on TensorE with the probability column as the (tiny)
stationary weight and V f16 as the moving operand, so each pair's output
lands as a PSUM row [1, head_dim] at partition 0 and is normalized
in-stream into a flat row buffer ([1, npairs*hd]) emitted with one DMA.

DMA transport (the kernel is DMA-engine bound; K+V = 64MiB/core):
- K rides the sync (SP) HWDGE queue in f32; V rides the gpsimd SWDGE
  queue cast f32->f16 in flight. This HWDGE+SWDGE mix keeps each SDMA
  engine's packets at ~280ns (~25 GB/s read); all-HWDGE configs cap
  every engine at ~21 GB/s regardless of ring count.
- The SWDGE descriptor rings throttle SDMA engine 15 ~17% below its
  peers. A DMA's partitions map to engines relative to its base
  (engine = partition//8 for full-width DMAs), so E15 serves partitions
  120-127. To offload it WITHOUT shrinking DMA lines (packet efficiency
  dies below ~8KB per partition line), 3 of every 16 pairs load as
  [120 partitions x 17 chunks] tiles (engines 0-14 only, 8.7KB lines);
  the rest are standard [128 x 16]. E15 ends up with 13/16 of a normal
  share, matching its speed deficit, and every DMA keeps full-size
  lines. The reduced pairs' 8 leftover cache rows ride a tiny batched
  sidecar (setup DMAs + one 8-deep matmul per reduced pair).
- The last pair's K tile loads early on the scalar (ACT) HWDGE ring, so
  after the final V packet only one PV matmul train + normalize + a
  16KB out DMA remain.
"""

import functools
import os
from contextlib import ExitStack

import numpy as np

import concourse.bacc as bacc
import concourse.bass as bass
import concourse.mybir as mybir
import concourse.tile as tile
from concourse import bass_utils

N_CORES = 8
P = 128   # partitions
NCH_S = 16   # chunks/partition, standard pairs (rows = p*16 + c)
NP_R = 120   # partitions used by reduced pairs
NCH_R = 17   # chunks/partition, reduced pairs (rows = p*17 + c)
# Pairs whose tiles skip engine 15 in the K (HWDGE) stream. V rides SWDGE,
# whose engine<->partition swizzle is interleaved (E15 <-> {92-95,124-127}),
# so reduced V tiles only drop half of E15's share - 4 of every 16 pairs
# reduced overshoots on K to compensate, balancing E15's total.
RED_MOD = (3, 7, 11, 15)

# test.py hooks: set TRACE=True before calling kernel() to collect a profile.
TRACE = False
TRACE_KWARGS = {}
LAST_RESULTS = None


def _reduced(npairs):
    return [p for p in range(npairs) if p % 16 in RED_MOD]


def _build_program(bl: int, n_heads: int, max_seq: int, hd: int, cache_len: int):
    """Build + compile the per-core program. bl = local batch count."""
    npairs = bl * n_heads
    assert hd == P
    assert cache_len == P * NCH_S == NP_R * NCH_R + 8
    sm_scale = 1.0 / float(np.sqrt(hd))
    reduced = _reduced(npairs)
    n_red = len(reduced)
    n_extra = cache_len - NP_R * NCH_R  # leftover rows per reduced pair (8)

    nc = bacc.Bacc("TRN2", target_bir_lowering=False, debug=False)
    f32 = mybir.dt.float32
    f16 = mybir.dt.float16

    kc = nc.dram_tensor("kc", [bl, n_heads, max_seq, hd], f32, kind="ExternalInput").ap()
    vc = nc.dram_tensor("vc", [bl, n_heads, max_seq, hd], f32, kind="ExternalInput").ap()
    q = nc.dram_tensor("q", [npairs, hd], f32, kind="ExternalInput").ap()
    kn = nc.dram_tensor("kn", [npairs, hd], f32, kind="ExternalInput").ap()
    vn = nc.dram_tensor("vn", [npairs, hd], f32, kind="ExternalInput").ap()
    ident = nc.dram_tensor("ident", [P, P], f32, kind="ExternalInput").ap()
    # sel[i, j] = 1 where j == reduced[i]: scatters the sidecar's
    # denominators into lrow columns with one matmul
    sel = nc.dram_tensor("sel", [n_red, npairs], f32, kind="ExternalInput").ap()
    out = nc.dram_tensor("out", [npairs, hd], f32, kind="ExternalOutput").ap()

    bh_stride = max_seq * hd

    with tile.TileContext(nc) as tc, ExitStack() as ctx:
        singles = ctx.enter_context(tc.tile_pool(name="singles", bufs=1))
        rows = ctx.enter_context(tc.tile_pool(name="rows", bufs=1))
        kpool = ctx.enter_context(tc.tile_pool(name="kpool", bufs=5))
        vpool = ctx.enter_context(tc.tile_pool(name="vpool", bufs=6))
        rkpool = ctx.enter_context(tc.tile_pool(name="rkpool", bufs=2))
        rvpool = ctx.enter_context(tc.tile_pool(name="rvpool", bufs=2))
        early_k = ctx.enter_context(tc.tile_pool(name="early_k", bufs=1))
        k16pool = ctx.enter_context(tc.tile_pool(name="k16pool", bufs=2))
        ppool = ctx.enter_context(tc.tile_pool(name="ppool", bufs=2))
        rppool = ctx.enter_context(tc.tile_pool(name="rppool", bufs=1))
        stats = ctx.enter_context(tc.tile_pool(name="stats", bufs=4))
        psum_o = ctx.enter_context(tc.tile_pool(name="psum_o", bufs=3, space="PSUM"))
        psum_q = ctx.enter_context(tc.tile_pool(name="psum_q", bufs=2, space="PSUM"))
        psum_1 = ctx.enter_context(tc.tile_pool(name="psum_1", bufs=1, space="PSUM"))

        def shape_of(p):
            return (NP_R, NCH_R) if p % 16 in RED_MOD else (P, NCH_S)

        def emit_k(p, engine=None, pool=None):
            np_, nch = shape_of(p)
            b, h = divmod(p, n_heads)
            kt = (pool or (rkpool if nch == NCH_R else kpool)).tile(
                [np_, nch, hd], f32, tag=f"kt{p}" if pool else "kt"
            )
            src = kc[b, h, 0 : np_ * nch, :].rearrange("(p c) d -> p c d", c=nch)
            (engine or nc.sync).dma_start(out=kt, in_=src)
            return kt

        def emit_v(p):
            # SWDGE casts V f32->f16 in flight (free on the DMA read side;
            # f16 weights/moving keep the PE matmuls at full rate)
            np_, nch = shape_of(p)
            b, h = divmod(p, n_heads)
            vt = (rvpool if nch == NCH_R else vpool).tile(
                [np_, nch, hd], f16, tag="vt"
            )
            src = vc[b, h, 0 : np_ * nch, :].rearrange("(p c) d -> p c d", c=nch)
            nc.gpsimd.dma_start(out=vt, in_=src)
            return vt

        # issue the first pairs' K loads before any setup traffic so the
        # sync ring's first instruction is a K DMA
        PRELOAD = min(3, npairs)
        kts = {p: emit_k(p) for p in range(PRELOAD)}
        vts = {p: emit_v(p) for p in range(PRELOAD)}

        ones_col = singles.tile([P, 1], f32)
        nc.vector.memset(ones_col, 1.0)

        # small setup loads lead the scalar (ACT) HWDGE ring
        def flat_row(t):
            return bass.AP(
                tensor=t.tensor, offset=t.offset, ap=[[0, 1], [1, npairs * hd]]
            )

        q_row = rows.tile([1, npairs * hd], f32, tag="row")
        nc.scalar.dma_start(out=q_row, in_=flat_row(q))
        vn_row = singles.tile([1, npairs * hd], f32)
        nc.scalar.dma_start(out=vn_row, in_=flat_row(vn))
        kn_all = singles.tile([npairs, hd], f32)
        nc.scalar.dma_start(out=kn_all, in_=kn)
        q_all = singles.tile([npairs, hd], f32)
        nc.scalar.dma_start(out=q_all, in_=q)
        ident_sb = singles.tile([P, P], f32)
        nc.scalar.dma_start(out=ident_sb, in_=ident)
        sel_sb = singles.tile([n_red, npairs], f32)
        nc.scalar.dma_start(out=sel_sb, in_=sel)

        # sidecar loads: the reduced pairs' leftover rows (2040..2047) and
        # their q rows, batched per block-of-16-pairs (uniform stride)
        e_row0 = NP_R * NCH_R
        ke2 = singles.tile([n_red, n_extra, hd], f32)
        ve2 = singles.tile([n_extra, n_red, hd], f32)
        q2 = singles.tile([n_red, hd], f32)
        nblk = npairs // 16
        per_blk = len(RED_MOD)
        for blk in range(nblk):
            i0 = blk * per_blk
            pair0 = blk * 16 + RED_MOD[0]
            pstride = RED_MOD[1] - RED_MOD[0]
            base = pair0 * bh_stride + e_row0 * hd
            nc.scalar.dma_start(
                out=ke2[i0 : i0 + per_blk, :, :],
                in_=bass.AP(
                    tensor=kc.tensor,
                    offset=kc.offset + base,
                    ap=[[pstride * bh_stride, per_blk], [hd, n_extra], [1, hd]],
                ),
            )
            nc.scalar.dma_start(
                out=ve2[:, i0 : i0 + per_blk, :],
                in_=bass.AP(
                    tensor=vc.tensor,
                    offset=vc.offset + base,
                    ap=[[hd, n_extra], [pstride * bh_stride, per_blk], [1, hd]],
                ),
            )
            nc.scalar.dma_start(
                out=q2[i0 : i0 + per_blk, :],
                in_=bass.AP(
                    tensor=q.tensor,
                    offset=q.offset + pair0 * hd,
                    ap=[[pstride * hd, per_blk], [1, hd]],
                ),
            )

        # the last pair's K tile, pinned, near the head of the scalar ring
        N_EARLY_K = 1 if npairs >= 8 else 0
        for p in range(npairs - N_EARLY_K, npairs):
            kts[p] = emit_k(p, engine=nc.scalar, pool=early_k)

        # all queries broadcast to every partition, once, as a PE outer
        # product ones[1,128] x q_row[1,*]. f16 replicas feed the 16-bit
        # score path.
        ones_row = singles.tile([1, P], f32)
        nc.vector.memset(ones_row, 1.0)
        q_all_b = singles.tile([P, npairs, hd], f16)
        GPAIRS = 4  # pairs per chunk; 4*hd f32 = one 2KB PSUM bank
        for g in range(npairs // GPAIRS):
            qb_ps = psum_q.tile([P, GPAIRS, hd], f32, tag="qb")
            qb_2d = bass.AP(
                tensor=qb_ps.tensor,
                offset=qb_ps.offset,
                ap=[qb_ps.ap[0], [1, GPAIRS * hd]],
            )
            nc.tensor.matmul(
                qb_2d,
                lhsT=ones_row,
                rhs=q_row[0:1, g * GPAIRS * hd : (g + 1) * GPAIRS * hd],
                start=True,
                stop=True,
            )
            nc.scalar.copy(q_all_b[:, g * GPAIRS : (g + 1) * GPAIRS, :], qb_ps)

        # Softmax denominators, one column per pair (partition 0).
        lrow = psum_1.tile([1, npairs], f32, tag="l")
        # Normalized output rows, all on partition 0, emitted with one DMA
        # (reuses q_row's slot - setup reads are done before pair 0 ends).
        final_row = rows.tile([1, npairs * hd], f32, tag="row")

        def bcast(ap2d, nb):
            return bass.AP(
                tensor=ap2d.tensor,
                offset=ap2d.offset,
                ap=[ap2d.ap[0], [0, nb], ap2d.ap[1]],
            )

        # ---- new-token scores, batched over all pairs, ending in a
        # partition-0 row p_newT so each pair's PV group can start with a
        # 1x1-weight matmul (PE requires base partition 0/32/64) ----
        prod_new = singles.tile([npairs, hd], f32)
        nc.vector.tensor_mul(prod_new, kn_all, q_all)
        s_new = singles.tile([npairs, 1], f32)
        nc.vector.reduce_sum(s_new, prod_new, axis=mybir.AxisListType.X)
        s_newT_ps = psum_1.tile([1, npairs], f32, tag="snT")
        nc.tensor.matmul(
            s_newT_ps, lhsT=s_new, rhs=ident_sb[:npairs, :npairs], start=True, stop=True
        )
        p_newT = singles.tile([1, npairs], f32)
        nc.scalar.activation(
            out=p_newT,
            in_=s_newT_ps,
            func=mybir.ActivationFunctionType.Exp,
            scale=sm_scale,
        )

        # lrow starts as p_newT (one 1x1 matmul); the sidecar and each pair
        # then accumulate their denominators (start=False)
        nc.tensor.matmul(
            lrow, lhsT=ones_col[0:1, 0:1], rhs=p_newT, start=True, stop=True
        )

        # ---- sidecar: scores/exp for the reduced pairs' leftover rows,
        # batched. Runs while the first K tiles stream in. ----
        prod_e = singles.tile([n_red, n_extra, hd], f32)
        nc.vector.tensor_mul(prod_e, ke2, bcast(q2, n_extra))
        s_e = singles.tile([n_red, n_extra], f32)
        nc.vector.reduce_sum(s_e, prod_e, axis=mybir.AxisListType.X)
        p_e = singles.tile([n_red, n_extra], f32)
        l_e = singles.tile([n_red, 1], f32)
        nc.scalar.activation(
            out=p_e,
            in_=s_e,
            func=mybir.ActivationFunctionType.Exp,
            scale=sm_scale,
            accum_out=l_e,
        )
        # sidecar denominators -> lrow columns (scatter-accumulate matmul)
        nc.tensor.matmul(lrow, lhsT=l_e, rhs=sel_sb, start=False, stop=True)
        # sidecar probabilities transposed to [n_extra, n_red] for the
        # per-pair PV matmuls
        pT_ps = psum_1.tile([n_extra, n_red], f32, tag="pT")
        nc.tensor.matmul(
            pT_ps, lhsT=p_e, rhs=ident_sb[:n_red, :n_red], start=True, stop=True
        )
        pT_sb = singles.tile([n_extra, n_red], f32)
        nc.scalar.copy(pT_sb, pT_ps)

        def emit_scores(p, kt):
            """f16 cast -> mul -> pairwise folds -> reduce -> exp -> l-MM."""
            np_, nch = shape_of(p)
            red = nch == NCH_R
            sfx = "r" if red else ""
            pp = rppool if red else ppool
            kt16 = k16pool.tile([np_, nch, hd], f16, tag="kt16" + sfx)
            nc.scalar.copy(kt16, kt)
            prod = pp.tile([np_, nch, hd], f16, tag="prod" + sfx)
            nc.vector.tensor_mul(prod, kt16, bcast(q_all_b[:np_, p, :], nch))
            fold1 = pp.tile([np_, nch, hd // 2], f16, tag="f1" + sfx)
            nc.vector.tensor_add(fold1, prod[:, :, : hd // 2], prod[:, :, hd // 2 :])
            fold2 = pp.tile([np_, nch, hd // 4], f16, tag="f2" + sfx)
            nc.vector.tensor_add(fold2, fold1[:, :, : hd // 4], fold1[:, :, hd // 4 :])
            s_tile = stats.tile([np_, nch], f32, tag="s" + sfx)
            nc.vector.reduce_sum(s_tile, fold2, axis=mybir.AxisListType.X)
            p_tile = stats.tile([np_, nch], f16, tag="p" + sfx)
            l_part = stats.tile([np_, 1], f32, tag="l" + sfx)
            nc.scalar.activation(
                out=p_tile,
                in_=s_tile,
                func=mybir.ActivationFunctionType.Exp,
                scale=sm_scale,
                accum_out=l_part,
            )
            nc.tensor.matmul(
                lrow[0:1, p : p + 1],
                lhsT=ones_col[:np_, :],
                rhs=l_part,
                start=False,
                stop=True,
            )
            return p_tile

        def emit_pv_norm(p, vt, p_tile):
            np_, nch = shape_of(p)
            acc_p = psum_o.tile([1, hd], f32, tag="acc")
            # new-token term starts the PV accumulation group
            nc.tensor.matmul(
                acc_p,
                lhsT=p_newT[0:1, p : p + 1],
                rhs=vn_row[0:1, p * hd : (p + 1) * hd],
                start=True,
                stop=False,
            )
            if nch == NCH_R:
                i = reduced.index(p)
                nc.tensor.matmul(
                    acc_p,
                    lhsT=pT_sb[:, i : i + 1],
                    rhs=ve2[:, i, :],
                    start=False,
                    stop=False,
                )
            # main PV train: probability column stationary, V f16 moving
            for c in range(nch):
                nc.tensor.matmul(
                    acc_p,
                    lhsT=p_tile[:, c : c + 1],
                    rhs=vt[:, c, :],
                    start=False,
                    stop=(c == nch - 1),
                )
            # per-pair normalize straight out of PSUM into the output row
            # buffer (runs mid-stream for every pair but the last)
            recip_p = stats.tile([1, 1], f32, tag="r")
            nc.vector.reciprocal(recip_p, lrow[0:1, p : p + 1])
            nc.scalar.mul(final_row[0:1, p * hd : (p + 1) * hd], acc_p, mul=recip_p)

        # For the last pairs, ALL score blocks are emitted before any PV /
        # normalize: otherwise each pair's reciprocal sits in the DVE queue
        # between score blocks and cascades V-arrival waits into the score
        # pipeline, stretching the tail.
        TAIL_T = min(4, npairs)
        for p in range(npairs):
            if p not in kts:
                kts[p] = emit_k(p)
            if p not in vts:
                vts[p] = emit_v(p)
            if p < npairs - TAIL_T:
                p_tile = emit_scores(p, kts[p])
                emit_pv_norm(p, vts[p], p_tile)
        tail_ptiles = {}
        for p in range(npairs - TAIL_T, npairs):
            tail_ptiles[p] = emit_scores(p, kts[p])
        for p in range(npairs - TAIL_T, npairs):
            emit_pv_norm(p, vts[p], tail_ptiles[p])

        # ---- emit: one 16KB DMA of all normalized rows ----
        out_flat = bass.AP(
            tensor=out.tensor, offset=out.offset, ap=[[0, 1], [1, npairs * hd]]
        )
        nc.scalar.dma_start(out=out_flat, in_=final_row)

    nc.compile()
    return nc


@functools.lru_cache(maxsize=4)
def _program(bl, n_heads, max_seq, hd, cache_len):
    return _build_program(bl, n_heads, max_seq, hd, cache_len)


def kernel(query, key, value, key_cache, value_cache, cache_len):
    global LAST_RESULTS
    query = np.asarray(query, dtype=np.float32)
    key = np.asarray(key, dtype=np.float32)
    value = np.asarray(value, dtype=np.float32)
    key_cache = np.asarray(key_cache, dtype=np.float32)
    value_cache = np.asarray(value_cache, dtype=np.float32)
    cache_len = int(cache_len)

    b_sz, q_len, d_model = query.shape
    _, n_heads, max_seq, hd = key_cache.shape
    assert q_len == 1 and d_model == n_heads * hd
    assert b_sz % N_CORES == 0
    bl = b_sz // N_CORES

    prog = _program(bl, n_heads, max_seq, hd, cache_len)

    npairs = bl * n_heads
    reduced = _reduced(npairs)
    ident = np.eye(P, dtype=np.float32)
    sel = np.zeros((len(reduced), npairs), dtype=np.float32)
    for i, p in enumerate(reduced):
        sel[i, p] = 1.0
    in_maps = []
    for i in range(N_CORES):
        sl = slice(i * bl, (i + 1) * bl)
        in_maps.append(
            {
                "kc": np.ascontiguousarray(key_cache[sl]),
                "vc": np.ascontiguousarray(value_cache[sl]),
                "q": np.ascontiguousarray(query[sl]).reshape(bl * n_heads, hd),
                "kn": np.ascontiguousarray(key[sl]).reshape(bl * n_heads, hd),
                "vn": np.ascontiguousarray(value[sl]).reshape(bl * n_heads, hd),
                "ident": ident,
                "sel": sel,
            }
        )

    try:
        res = bass_utils.run_bass_kernel_spmd(
            prog, in_maps, core_ids=list(range(N_CORES)), trace=TRACE, **TRACE_KWARGS
        )
    except Exception:
        # A previously crashed NeuronCore can leave the first execution
        # attempt failing with a transient runtime error; retry once.
        res = bass_utils.run_bass_kernel_spmd(
            prog, in_maps, core_ids=list(range(N_CORES)), trace=TRACE, **TRACE_KWARGS
        )
    LAST_RESULTS = res
    outs = [res.results[i]["out"].reshape(bl, q_len, d_model) for i in range(N_CORES)]
    return np.concatenate(outs, axis=0)


# revision 53
# speedup vs baseline: 2.0779x; 1.0179x over previous
"""Decode-step multi-head attention with KV cache (DeepSpeed-inference style).

Full shapes (hardcoded per problem spec):
  query/key/value: [16, 1, 2048] f32
  key_cache/value_cache: [16, 16, 4096, 128] f32
  cache_len: scalar int (2048)
Output: [16, 1, 2048] f32

Strategy: data-parallel over batch across 8 NeuronCores (2 batches/core =
32 (batch, head) pairs per core). Per pair, the core streams the K and V
cache slices from HBM, computes scores with multiply+reduce on VectorE,
exp via ScalarE (fused row-sum for the softmax denominator), and runs the
PV reduction on TensorE with the probability column as the (tiny)
stationary weight and V f16 as the moving operand, so each pair's output
lands as a PSUM row [1, head_dim] at partition 0 and is normalized
in-stream into a flat row buffer ([1, npairs*hd]) emitted with one DMA.

DMA transport (the kernel is DMA-engine bound; K+V = 64MiB/core):
- K rides the sync (SP) HWDGE queue in f32; V rides the gpsimd SWDGE
  queue cast f32->f16 in flight. This HWDGE+SWDGE mix keeps each SDMA
  engine's packets at ~280ns (~25 GB/s read); all-HWDGE configs cap
  every engine at ~21 GB/s regardless of ring count.
- The SWDGE descriptor rings throttle SDMA engine 15 ~17% below its
  peers. A DMA's partitions map to engines relative to its base
  (engine = partition//8 for full-width DMAs), so E15 serves partitions
  120-127. To offload it WITHOUT shrinking DMA lines (packet efficiency
  dies below ~8KB per partition line), 3 of every 16 pairs load as
  [120 partitions x 17 chunks] tiles (engines 0-14 only, 8.7KB lines);
  the rest are standard [128 x 16]. E15 ends up with 13/16 of a normal
  share, matching its speed deficit, and every DMA keeps full-size
  lines. The reduced pairs' 8 leftover cache rows ride a tiny batched
  sidecar (setup DMAs + one 8-deep matmul per reduced pair).
- The last pair's K tile loads early on the scalar (ACT) HWDGE ring, so
  after the final V packet only one PV matmul train + normalize + a
  16KB out DMA remain.
"""

import functools
import os
from contextlib import ExitStack

import numpy as np

import concourse.bacc as bacc
import concourse.bass as bass
import concourse.mybir as mybir
import concourse.tile as tile
from concourse import bass_utils

N_CORES = 8
P = 128   # partitions
NCH_S = 16   # chunks/partition, standard pairs (rows = p*16 + c)
NP_R = 120   # partitions used by reduced pairs
NCH_R = 17   # chunks/partition, reduced pairs (rows = p*17 + c)
# Pairs whose tiles skip engine 15 in the K (HWDGE) stream. V rides SWDGE,
# whose engine<->partition swizzle is interleaved (E15 <-> {92-95,124-127}),
# so reduced V tiles only drop half of E15's share - 4 of every 16 pairs
# reduced overshoots on K to compensate, balancing E15's total.
RED_MOD = (3, 7, 11, 15)

# test.py hooks: set TRACE=True before calling kernel() to collect a profile.
TRACE = False
TRACE_KWARGS = {}
LAST_RESULTS = None


def _reduced(npairs):
    return [p for p in range(npairs) if p % 16 in RED_MOD]


def _build_program(bl: int, n_heads: int, max_seq: int, hd: int, cache_len: int):
    """Build + compile the per-core program. bl = local batch count."""
    npairs = bl * n_heads
    assert hd == P
    assert cache_len == P * NCH_S == NP_R * NCH_R + 8
    sm_scale = 1.0 / float(np.sqrt(hd))
    reduced = _reduced(npairs)
    n_red = len(reduced)
    n_extra = cache_len - NP_R * NCH_R  # leftover rows per reduced pair (8)

    nc = bacc.Bacc("TRN2", target_bir_lowering=False, debug=False)
    f32 = mybir.dt.float32
    f16 = mybir.dt.float16

    kc = nc.dram_tensor("kc", [bl, n_heads, max_seq, hd], f32, kind="ExternalInput").ap()
    vc = nc.dram_tensor("vc", [bl, n_heads, max_seq, hd], f32, kind="ExternalInput").ap()
    q = nc.dram_tensor("q", [npairs, hd], f32, kind="ExternalInput").ap()
    kn = nc.dram_tensor("kn", [npairs, hd], f32, kind="ExternalInput").ap()
    vn = nc.dram_tensor("vn", [npairs, hd], f32, kind="ExternalInput").ap()
    ident = nc.dram_tensor("ident", [P, P], f32, kind="ExternalInput").ap()
    # sel[i, j] = 1 where j == reduced[i]: scatters the sidecar's
    # denominators into lrow columns with one matmul
    sel = nc.dram_tensor("sel", [n_red, npairs], f32, kind="ExternalInput").ap()
    out = nc.dram_tensor("out", [npairs, hd], f32, kind="ExternalOutput").ap()

    bh_stride = max_seq * hd

    with tile.TileContext(nc) as tc, ExitStack() as ctx:
        singles = ctx.enter_context(tc.tile_pool(name="singles", bufs=1))
        rows = ctx.enter_context(tc.tile_pool(name="rows", bufs=1))
        kpool = ctx.enter_context(tc.tile_pool(name="kpool", bufs=5))
        vpool = ctx.enter_context(tc.tile_pool(name="vpool", bufs=7))
        rkpool = ctx.enter_context(tc.tile_pool(name="rkpool", bufs=2))
        rvpool = ctx.enter_context(tc.tile_pool(name="rvpool", bufs=2))
        early_k = ctx.enter_context(tc.tile_pool(name="early_k", bufs=1))
        k16pool = ctx.enter_context(tc.tile_pool(name="k16pool", bufs=2))
        ppool = ctx.enter_context(tc.tile_pool(name="ppool", bufs=2))
        rppool = ctx.enter_context(tc.tile_pool(name="rppool", bufs=1))
        stats = ctx.enter_context(tc.tile_pool(name="stats", bufs=4))
        psum_o = ctx.enter_context(tc.tile_pool(name="psum_o", bufs=4, space="PSUM"))
        psum_q = ctx.enter_context(tc.tile_pool(name="psum_q", bufs=2, space="PSUM"))
        psum_1 = ctx.enter_context(tc.tile_pool(name="psum_1", bufs=1, space="PSUM"))

        def shape_of(p):
            return (NP_R, NCH_R) if p % 16 in RED_MOD else (P, NCH_S)

        def emit_k(p, engine=None, pool=None):
            np_, nch = shape_of(p)
            b, h = divmod(p, n_heads)
            kt = (pool or (rkpool if nch == NCH_R else kpool)).tile(
                [np_, nch, hd], f32, tag=f"kt{p}" if pool else "kt"
            )
            src = kc[b, h, 0 : np_ * nch, :].rearrange("(p c) d -> p c d", c=nch)
            (engine or nc.sync).dma_start(out=kt, in_=src)
            return kt

        def emit_v(p):
            # SWDGE casts V f32->f16 in flight (free on the DMA read side;
            # f16 weights/moving keep the PE matmuls at full rate)
            np_, nch = shape_of(p)
            b, h = divmod(p, n_heads)
            vt = (rvpool if nch == NCH_R else vpool).tile(
                [np_, nch, hd], f16, tag="vt"
            )
            src = vc[b, h, 0 : np_ * nch, :].rearrange("(p c) d -> p c d", c=nch)
            nc.gpsimd.dma_start(out=vt, in_=src)
            return vt

        # issue the first pairs' K loads before any setup traffic so the
        # sync ring's first instruction is a K DMA
        PRELOAD = min(3, npairs)
        kts = {p: emit_k(p) for p in range(PRELOAD)}
        vts = {p: emit_v(p) for p in range(PRELOAD)}

        ones_col = singles.tile([P, 1], f32)
        nc.vector.memset(ones_col, 1.0)

        # small setup loads lead the scalar (ACT) HWDGE ring
        def flat_row(t):
            return bass.AP(
                tensor=t.tensor, offset=t.offset, ap=[[0, 1], [1, npairs * hd]]
            )

        q_row = rows.tile([1, npairs * hd], f32, tag="row")
        nc.scalar.dma_start(out=q_row, in_=flat_row(q))
        vn_row = singles.tile([1, npairs * hd], f32)
        nc.scalar.dma_start(out=vn_row, in_=flat_row(vn))
        kn_all = singles.tile([npairs, hd], f32)
        nc.scalar.dma_start(out=kn_all, in_=kn)
        q_all = singles.tile([npairs, hd], f32)
        nc.scalar.dma_start(out=q_all, in_=q)
        ident_sb = singles.tile([P, P], f32)
        nc.scalar.dma_start(out=ident_sb, in_=ident)
        sel_sb = singles.tile([n_red, npairs], f32)
        nc.scalar.dma_start(out=sel_sb, in_=sel)

        # sidecar loads: the reduced pairs' leftover rows (2040..2047) and
        # their q rows, batched per block-of-16-pairs (uniform stride)
        e_row0 = NP_R * NCH_R
        ke2 = singles.tile([n_red, n_extra, hd], f32)
        ve2 = singles.tile([n_extra, n_red, hd], f32)
        q2 = singles.tile([n_red, hd], f32)
        nblk = npairs // 16
        per_blk = len(RED_MOD)
        for blk in range(nblk):
            i0 = blk * per_blk
            pair0 = blk * 16 + RED_MOD[0]
            pstride = RED_MOD[1] - RED_MOD[0]
            base = pair0 * bh_stride + e_row0 * hd
            nc.scalar.dma_start(
                out=ke2[i0 : i0 + per_blk, :, :],
                in_=bass.AP(
                    tensor=kc.tensor,
                    offset=kc.offset + base,
                    ap=[[pstride * bh_stride, per_blk], [hd, n_extra], [1, hd]],
                ),
            )
            nc.scalar.dma_start(
                out=ve2[:, i0 : i0 + per_blk, :],
                in_=bass.AP(
                    tensor=vc.tensor,
                    offset=vc.offset + base,
                    ap=[[hd, n_extra], [pstride * bh_stride, per_blk], [1, hd]],
                ),
            )
            nc.scalar.dma_start(
                out=q2[i0 : i0 + per_blk, :],
                in_=bass.AP(
                    tensor=q.tensor,
                    offset=q.offset + pair0 * hd,
                    ap=[[pstride * hd, per_blk], [1, hd]],
                ),
            )

        # the last pair's K tile, pinned, near the head of the scalar ring
        N_EARLY_K = 1 if npairs >= 8 else 0
        for p in range(npairs - N_EARLY_K, npairs):
            kts[p] = emit_k(p, engine=nc.scalar, pool=early_k)

        # all queries broadcast to every partition, once, as a PE outer
        # product ones[1,128] x q_row[1,*]. f16 replicas feed the 16-bit
        # score path.
        ones_row = singles.tile([1, P], f32)
        nc.vector.memset(ones_row, 1.0)
        q_all_b = singles.tile([P, npairs, hd], f16)
        GPAIRS = 4  # pairs per chunk; 4*hd f32 = one 2KB PSUM bank
        for g in range(npairs // GPAIRS):
            qb_ps = psum_q.tile([P, GPAIRS, hd], f32, tag="qb")
            qb_2d = bass.AP(
                tensor=qb_ps.tensor,
                offset=qb_ps.offset,
                ap=[qb_ps.ap[0], [1, GPAIRS * hd]],
            )
            nc.tensor.matmul(
                qb_2d,
                lhsT=ones_row,
                rhs=q_row[0:1, g * GPAIRS * hd : (g + 1) * GPAIRS * hd],
                start=True,
                stop=True,
            )
            nc.scalar.copy(q_all_b[:, g * GPAIRS : (g + 1) * GPAIRS, :], qb_ps)

        # Softmax denominators, one column per pair (partition 0).
        lrow = psum_1.tile([1, npairs], f32, tag="l")
        # Normalized output rows, all on partition 0, emitted with one DMA
        # (reuses q_row's slot - setup reads are done before pair 0 ends).
        final_row = rows.tile([1, npairs * hd], f32, tag="row")

        def bcast(ap2d, nb):
            return bass.AP(
                tensor=ap2d.tensor,
                offset=ap2d.offset,
                ap=[ap2d.ap[0], [0, nb], ap2d.ap[1]],
            )

        # ---- new-token scores, batched over all pairs, ending in a
        # partition-0 row p_newT so each pair's PV group can start with a
        # 1x1-weight matmul (PE requires base partition 0/32/64) ----
        prod_new = singles.tile([npairs, hd], f32)
        nc.vector.tensor_mul(prod_new, kn_all, q_all)
        s_new = singles.tile([npairs, 1], f32)
        nc.vector.reduce_sum(s_new, prod_new, axis=mybir.AxisListType.X)
        # s_new^T stages through the lrow bank (PSUM banks are fully booked)
        nc.tensor.matmul(
            lrow, lhsT=s_new, rhs=ident_sb[:npairs, :npairs], start=True, stop=True
        )
        p_newT = singles.tile([1, npairs], f32)
        nc.scalar.activation(
            out=p_newT,
            in_=lrow,
            func=mybir.ActivationFunctionType.Exp,
            scale=sm_scale,
        )

        # lrow starts as p_newT (one 1x1 matmul); the sidecar and each pair
        # then accumulate their denominators (start=False)
        nc.tensor.matmul(
            lrow, lhsT=ones_col[0:1, 0:1], rhs=p_newT, start=True, stop=True
        )

        # ---- sidecar: scores/exp for the reduced pairs' leftover rows,
        # batched. Runs while the first K tiles stream in. ----
        prod_e = singles.tile([n_red, n_extra, hd], f32)
        nc.vector.tensor_mul(prod_e, ke2, bcast(q2, n_extra))
        s_e = singles.tile([n_red, n_extra], f32)
        nc.vector.reduce_sum(s_e, prod_e, axis=mybir.AxisListType.X)
        p_e = singles.tile([n_red, n_extra], f32)
        l_e = singles.tile([n_red, 1], f32)
        nc.scalar.activation(
            out=p_e,
            in_=s_e,
            func=mybir.ActivationFunctionType.Exp,
            scale=sm_scale,
            accum_out=l_e,
        )
        # sidecar denominators -> lrow columns (scatter-accumulate matmul)
        nc.tensor.matmul(lrow, lhsT=l_e, rhs=sel_sb, start=False, stop=True)
        # sidecar probabilities transposed to [n_extra, n_red] for the
        # per-pair PV matmuls
        pT_ps = psum_1.tile([n_extra, n_red], f32, tag="pT")
        nc.tensor.matmul(
            pT_ps, lhsT=p_e, rhs=ident_sb[:n_red, :n_red], start=True, stop=True
        )
        pT_sb = singles.tile([n_extra, n_red], f32)
        nc.scalar.copy(pT_sb, pT_ps)

        def emit_scores(p, kt):
            """f16 cast -> mul -> pairwise folds -> reduce -> exp -> l-MM."""
            np_, nch = shape_of(p)
            red = nch == NCH_R
            sfx = "r" if red else ""
            pp = rppool if red else ppool
            kt16 = k16pool.tile([np_, nch, hd], f16, tag="kt16" + sfx)
            nc.scalar.copy(kt16, kt)
            prod = pp.tile([np_, nch, hd], f16, tag="prod" + sfx)
            nc.vector.tensor_mul(prod, kt16, bcast(q_all_b[:np_, p, :], nch))
            fold1 = pp.tile([np_, nch, hd // 2], f16, tag="f1" + sfx)
            nc.vector.tensor_add(fold1, prod[:, :, : hd // 2], prod[:, :, hd // 2 :])
            fold2 = pp.tile([np_, nch, hd // 4], f16, tag="f2" + sfx)
            nc.vector.tensor_add(fold2, fold1[:, :, : hd // 4], fold1[:, :, hd // 4 :])
            s_tile = stats.tile([np_, nch], f32, tag="s" + sfx)
            nc.vector.reduce_sum(s_tile, fold2, axis=mybir.AxisListType.X)
            p_tile = stats.tile([np_, nch], f16, tag="p" + sfx)
            l_part = stats.tile([np_, 1], f32, tag="l" + sfx)
            nc.scalar.activation(
                out=p_tile,
                in_=s_tile,
                func=mybir.ActivationFunctionType.Exp,
                scale=sm_scale,
                accum_out=l_part,
            )
            nc.tensor.matmul(
                lrow[0:1, p : p + 1],
                lhsT=ones_col[:np_, :],
                rhs=l_part,
                start=False,
                stop=True,
            )
            return p_tile

        def emit_pv_norm(p, vt, p_tile):
            np_, nch = shape_of(p)
            acc_p = psum_o.tile([1, hd], f32, tag="acc")
            # new-token term starts the PV accumulation group
            nc.tensor.matmul(
                acc_p,
                lhsT=p_newT[0:1, p : p + 1],
                rhs=vn_row[0:1, p * hd : (p + 1) * hd],
                start=True,
                stop=False,
            )
            if nch == NCH_R:
                i = reduced.index(p)
                nc.tensor.matmul(
                    acc_p,
                    lhsT=pT_sb[:, i : i + 1],
                    rhs=ve2[:, i, :],
                    start=False,
                    stop=False,
                )
            # main PV train: probability column stationary, V f16 moving
            for c in range(nch):
                nc.tensor.matmul(
                    acc_p,
                    lhsT=p_tile[:, c : c + 1],
                    rhs=vt[:, c, :],
                    start=False,
                    stop=(c == nch - 1),
                )
            # per-pair normalize straight out of PSUM into the output row
            # buffer (runs mid-stream for every pair but the last)
            recip_p = stats.tile([1, 1], f32, tag="r")
            nc.vector.reciprocal(recip_p, lrow[0:1, p : p + 1])
            nc.scalar.mul(final_row[0:1, p * hd : (p + 1) * hd], acc_p, mul=recip_p)

        # For the last pairs, ALL score blocks are emitted before any PV /
        # normalize: otherwise each pair's reciprocal sits in the DVE queue
        # between score blocks and cascades V-arrival waits into the score
        # pipeline, stretching the tail. The V ring also ends with the pair
        # whose PV train is emitted LAST (here: the second-to-last pair,
        # swapped to the end), so the final train never waits mid-queue.
        TAIL_T = min(4, npairs)
        v_order = list(range(npairs))
        tail_order = list(range(npairs - TAIL_T, npairs))
        if TAIL_T >= 2:
            tail_order[-1], tail_order[-2] = tail_order[-2], tail_order[-1]
            v_order[-1], v_order[-2] = v_order[-2], v_order[-1]
        for idx in range(npairs):
            if idx not in kts:
                kts[idx] = emit_k(idx)
            pv = v_order[idx]
            if pv not in vts:
                vts[pv] = emit_v(pv)
            if idx < npairs - TAIL_T:
                p_tile = emit_scores(idx, kts[idx])
                emit_pv_norm(idx, vts[idx], p_tile)
        tail_ptiles = {}
        for p in tail_order:
            tail_ptiles[p] = emit_scores(p, kts[p])
        for p in tail_order:
            emit_pv_norm(p, vts[p], tail_ptiles[p])

        # ---- emit: one 16KB DMA of all normalized rows ----
        out_flat = bass.AP(
            tensor=out.tensor, offset=out.offset, ap=[[0, 1], [1, npairs * hd]]
        )
        nc.scalar.dma_start(out=out_flat, in_=final_row)

    nc.compile()
    return nc


@functools.lru_cache(maxsize=4)
def _program(bl, n_heads, max_seq, hd, cache_len):
    return _build_program(bl, n_heads, max_seq, hd, cache_len)


def kernel(query, key, value, key_cache, value_cache, cache_len):
    global LAST_RESULTS
    query = np.asarray(query, dtype=np.float32)
    key = np.asarray(key, dtype=np.float32)
    value = np.asarray(value, dtype=np.float32)
    key_cache = np.asarray(key_cache, dtype=np.float32)
    value_cache = np.asarray(value_cache, dtype=np.float32)
    cache_len = int(cache_len)

    b_sz, q_len, d_model = query.shape
    _, n_heads, max_seq, hd = key_cache.shape
    assert q_len == 1 and d_model == n_heads * hd
    assert b_sz % N_CORES == 0
    bl = b_sz // N_CORES

    prog = _program(bl, n_heads, max_seq, hd, cache_len)

    npairs = bl * n_heads
    reduced = _reduced(npairs)
    ident = np.eye(P, dtype=np.float32)
    sel = np.zeros((len(reduced), npairs), dtype=np.float32)
    for i, p in enumerate(reduced):
        sel[i, p] = 1.0
    in_maps = []
    for i in range(N_CORES):
        sl = slice(i * bl, (i + 1) * bl)
        in_maps.append(
            {
                "kc": np.ascontiguousarray(key_cache[sl]),
                "vc": np.ascontiguousarray(value_cache[sl]),
                "q": np.ascontiguousarray(query[sl]).reshape(bl * n_heads, hd),
                "kn": np.ascontiguousarray(key[sl]).reshape(bl * n_heads, hd),
                "vn": np.ascontiguousarray(value[sl]).reshape(bl * n_heads, hd),
                "ident": ident,
                "sel": sel,
            }
        )

    try:
        res = bass_utils.run_bass_kernel_spmd(
            prog, in_maps, core_ids=list(range(N_CORES)), trace=TRACE, **TRACE_KWARGS
        )
    except Exception:
        # A previously crashed NeuronCore can leave the first execution
        # attempt failing with a transient runtime error; retry once.
        res = bass_utils.run_bass_kernel_spmd(
            prog, in_maps, core_ids=list(range(N_CORES)), trace=TRACE, **TRACE_KWARGS
        )
    LAST_RESULTS = res
    outs = [res.results[i]["out"].reshape(bl, q_len, d_model) for i in range(N_CORES)]
    return np.concatenate(outs, axis=0)
